# revision 1
# baseline (speedup 1.0000x reference)
"""Trainium2 Bass kernel for nn_MultiHeadAttention_58548994179754.

Sharding: 8 cores = 2 batches x 4 head-groups (4 heads of 64 dims each per core).
Per core:
  - xqT [256,2048] = (Wq_g @ q[b]^T)*SCALE + bq_g*SCALE   (transposed layout; bias via an
    augmented ones-row in the contraction)
  - xkT [256,2048] likewise for the K rows of Wkv
  - xv  [2048,256] in natural layout plus a ones column per head (softmax denominator)
  - per head: logitsT[k,q] matmul; u = exp(logitsT + maskf[k]) (mask = per-partition ACT
    bias); attn_unnormT[d,q] + denom row = xv_aug^T @ u accumulated over k-tiles;
    normalize by 1/(denom+1)  [add-one softmax with shift m=0, mathematically identical
    to the reference's max-shift]
  - out_partial [2048,1024] = attn_core @ Wo_g^T (row-slice of the O-projection)
Host: out[b] = sum of the 4 cores' partials + bo.

All matmul operands are float32r (single-pass fp32 on the PE, 4x the fp32 rate;
HW-measured final rel err ~2.6e-4 vs the f64 oracle).  Even/odd heads of a pair sit on
PE row-groups 0-1/2-3 so their K=64 QK matmuls can overlap in the array.  The optional
`repeat` build wraps the whole body in an on-device loop for wall-clock-diff timing.
"""

import os
import sys

import numpy as np

sys.path.insert(0, "/opt/trn_rl_repo")

B, N, E = 2, 2048, 1024
H, D = 16, 64
HPC = 4  # heads per core
DPC = HPC * D  # 256 output dims per core
SCALE = float(E) ** -0.5
KC = 9  # contraction chunks: 8x128 data + 1 chunk (bias/ones row + zero pad)
CPAD = KC * 128  # 1152
NT = N // 128  # 16 seq tiles
MASK_NEG = np.float32(-1e30)

_CACHED = {}


def build_bass(repeat=1):
    import concourse.bass as bass
    import concourse.mybir as mybir
    import concourse.tile as tile
    from concourse import bacc

    f32 = mybir.dt.float32
    f32r = mybir.dt.float32r
    Exp = mybir.ActivationFunctionType.Exp

    nc = bacc.Bacc("TRN2", target_bir_lowering=False)

    qT = nc.dram_tensor("qT", (KC, 128, N), f32r, kind="ExternalInput")
    kvT = nc.dram_tensor("kvT", (KC, 128, N), f32r, kind="ExternalInput")
    wq = nc.dram_tensor("wq", (KC, 128, DPC), f32r, kind="ExternalInput")
    wk = nc.dram_tensor("wk", (KC, 128, DPC), f32r, kind="ExternalInput")
    wv = nc.dram_tensor("wv", (KC, 128, DPC), f32r, kind="ExternalInput")
    wo = nc.dram_tensor("wo", (2, 128, E), f32r, kind="ExternalInput")
    mk = nc.dram_tensor("mk", (128, NT), f32, kind="ExternalInput")
    ones1 = nc.dram_tensor("ones1", (128, NT, HPC, 1), f32r, kind="ExternalInput")
    outp = nc.dram_tensor("outp", (N, E), f32, kind="ExternalOutput")

    with tile.TileContext(nc) as tc:
        with (
            tc.tile_pool(name="const", bufs=1) as const,
            tc.tile_pool(name="io", bufs=3) as io_pool,
            tc.tile_pool(name="ups", bufs=4) as upool,
            tc.tile_pool(name="rps", bufs=2) as rpool,
            tc.tile_pool(name="osb", bufs=2) as osb,
            tc.tile_pool(name="ps", bufs=4, space="PSUM") as ps,
        ):
            wq_sb = const.tile([128, KC, DPC], f32r, name="wq_sb")
            wk_sb = const.tile([128, KC, DPC], f32r, name="wk_sb")
            wv_sb = const.tile([128, KC, DPC], f32r, name="wv_sb")
            wo_sb = const.tile([128, 2, E], f32r, name="wo_sb")
            mk_sb = const.tile([128, NT], f32, name="mk_sb")
            xqT = const.tile([128, 2, N], f32r, name="xqT")
            xkT = const.tile([128, 2, N], f32r, name="xkT")
            xv = const.tile([128, NT, HPC, D + 1], f32r, name="xv")
            attn = const.tile([128, 2, N], f32r, name="attn")
            warm = const.tile([1, 8], f32, name="warm")

            for kc in range(KC):
                nc.sync.dma_start(wq_sb[:, kc], wq[kc])
            nc.sync.dma_start(mk_sb[:], mk[:])
            # ones columns for the softmax-denominator rows
            nc.sync.dma_start(xv[:, :, :, D : D + 1], ones1[:])
            # warm up the ACT exp table early so the table load overlaps phase A
            nc.vector.memset(warm[:], 0.0)
            nc.scalar.activation(warm[:], warm[:], Exp)

            def body(_iv=None):
                # ---------------- Phase A: projections ----------------
                # prefetch kvT half-0 so the K/V phase starts right after Q
                kv_half0 = io_pool.tile(
                    [128, KC, 1024], f32r, tag="kvres", name="kv_sb0", bufs=1
                )
                for kc in range(KC):
                    nc.sync.dma_start(kv_half0[:, kc], kvT[kc, :, 0:1024])
                # Q: two halves of n; two 128-row m-tiles per half
                for half in range(2):
                    hsl = slice(half * 1024, (half + 1) * 1024)
                    qps = [
                        ps.tile([128, 1024], f32, tag="ps", name=f"qps{half}_{m}")
                        for m in range(2)
                    ]
                    for kc in range(KC):
                        qt = io_pool.tile([128, 1024], f32r, tag="io", name="qt")
                        nc.sync.dma_start(qt[:], qT[kc, :, hsl])
                        for m in range(2):
                            for g in range(2):
                                nc.tensor.matmul(
                                    qps[m][:, g * 512 : (g + 1) * 512],
                                    wq_sb[:, kc, m * 128 : (m + 1) * 128],
                                    qt[:, g * 512 : (g + 1) * 512],
                                    start=(kc == 0),
                                    stop=(kc == KC - 1),
                                )
                    for m in range(2):
                        nc.vector.tensor_copy(xqT[:, m, hsl], qps[m][:])
                # K and V share a fully-resident kvT half
                for kc in range(KC):
                    nc.sync.dma_start(wk_sb[:, kc], wk[kc])
                    nc.sync.dma_start(wv_sb[:, kc], wv[kc])
                for half in range(2):
                    hsl = slice(half * 1024, (half + 1) * 1024)
                    if half == 0:
                        kv_sb = kv_half0
                    else:
                        kv_sb = io_pool.tile(
                            [128, KC, 1024], f32r, tag="kvres", name="kv_sb1", bufs=1
                        )
                        for kc in range(KC):
                            nc.sync.dma_start(kv_sb[:, kc], kvT[kc, :, hsl])
                    kps = [
                        ps.tile([128, 1024], f32, tag="ps", name=f"kps{half}_{m}")
                        for m in range(2)
                    ]
                    for kc in range(KC):
                        for m in range(2):
                            for g in range(2):
                                nc.tensor.matmul(
                                    kps[m][:, g * 512 : (g + 1) * 512],
                                    wk_sb[:, kc, m * 128 : (m + 1) * 128],
                                    kv_sb[:, kc, g * 512 : (g + 1) * 512],
                                    start=(kc == 0),
                                    stop=(kc == KC - 1),
                                )
                    for m in range(2):
                        nc.vector.tensor_copy(xkT[:, m, hsl], kps[m][:])
                    for nt in range(8):
                        gnt = half * 8 + nt
                        vp = ps.tile([128, 256], f32, tag="ps", name=f"vp{half}_{nt}")
                        for kc in range(KC):
                            nc.tensor.matmul(
                                vp[:],
                                kv_sb[:, kc, nt * 128 : (nt + 1) * 128],
                                wv_sb[:, kc, :],
                                start=(kc == 0),
                                stop=(kc == KC - 1),
                            )
                        for h in range(HPC):
                            nc.vector.tensor_copy(
                                xv[:, gnt, h, 0:D], vp[:, h * 64 : (h + 1) * 64]
                            )

                # ---------------- Phase B: attention ----------------
                nc.sync.dma_start(wo_sb[:, 0], wo[0])
                nc.sync.dma_start(wo_sb[:, 1], wo[1])
                for qb in range(2):  # 1024-wide query blocks
                    for hp in range(2):  # head pair (chunk of xqT/xkT partitions)
                        avs = [
                            ps.tile([65, 1024], f32, tag="ps", name=f"av{hp}{qb}_{i}")
                            for i in range(2)
                        ]
                        for kt in range(NT):
                            us = []
                            for h2 in range(2):
                                po = 64 * h2
                                qk = ps.tile(
                                    [128, 1024], f32, tag="ps", name=f"qk{h2}"
                                )
                                for s in range(2):
                                    nc.tensor.matmul(
                                        qk[:, s * 512 : (s + 1) * 512],
                                        xkT[po : po + 64, hp, kt * 128 : (kt + 1) * 128],
                                        xqT[
                                            po : po + 64,
                                            hp,
                                            qb * 1024 + s * 512 : qb * 1024 + (s + 1) * 512,
                                        ],
                                        start=True,
                                        stop=True,
                                    )
                                u = upool.tile([128, 1024], f32r, tag="u", name=f"u{h2}")
                                nc.scalar.activation(
                                    u[:], qk[:], Exp, bias=mk_sb[:, kt : kt + 1], scale=1.0
                                )
                                us.append(u)
                            for h2 in range(2):
                                h = 2 * hp + h2
                                for s in range(2):
                                    nc.tensor.matmul(
                                        avs[h2][:, s * 512 : (s + 1) * 512],
                                        xv[:, kt, h, :],
                                        us[h2][:, s * 512 : (s + 1) * 512],
                                        start=(kt == 0),
                                        stop=(kt == NT - 1),
                                    )
                        for h2 in range(2):
                            av = avs[h2]
                            po = 64 * h2
                            rs = rpool.tile([1, 1024], f32, tag="rs", name="rs")
                            nc.vector.tensor_scalar_add(rs[:], av[64:65, :], 1.0)
                            nc.vector.reciprocal(rs[:], rs[:])
                            rb = rpool.tile([64, 1024], f32, tag="rb", name="rb")
                            nc.gpsimd.partition_broadcast(rb[:], rs[0:1, :])
                            nc.vector.tensor_mul(
                                attn[po : po + 64, hp, qb * 1024 : (qb + 1) * 1024],
                                av[0:64, :],
                                rb[:],
                            )

                    # ------------ Phase C: output projection for this qb ------------
                    for nt in range(qb * 8, qb * 8 + 8):
                        op = ps.tile([128, 1024], f32, tag="ps", name="op")
                        for c in range(2):
                            for s in range(2):
                                nc.tensor.matmul(
                                    op[:, s * 512 : (s + 1) * 512],
                                    attn[:, c, nt * 128 : (nt + 1) * 128],
                                    wo_sb[:, c, s * 512 : (s + 1) * 512],
                                    start=(c == 0),
                                    stop=(c == 1),
                                )
                        ot = osb.tile([128, 1024], f32, tag="ot", name="ot")
                        nc.vector.tensor_copy(ot[:], op[:])
                        nc.sync.dma_start(outp[nt * 128 : (nt + 1) * 128, :], ot[:])

            if repeat == 1:
                body()
            else:
                with tc.For_i(0, repeat, 1) as _i:
                    body(_i)

    nc.compile()
    return nc


def make_in_maps(q, kv, mask, Wq, bq, Wkv, bkv, Wo, bo):
    q = np.asarray(q, dtype=np.float32)
    kv = np.asarray(kv, dtype=np.float32)
    mask = np.asarray(mask)
    Wq = np.asarray(Wq, dtype=np.float32)
    bq = np.asarray(bq, dtype=np.float32)
    Wkv = np.asarray(Wkv, dtype=np.float32)
    bkv = np.asarray(bkv, dtype=np.float32)
    Wo = np.asarray(Wo, dtype=np.float32)

    Wk, Wv = Wkv[:E], Wkv[E:]
    bk, bv = bkv[:E], bkv[E:]

    qTa = {}
    kvTa = {}
    mks = {}
    for b in range(B):
        t = np.zeros((CPAD, N), np.float32)
        t[:E] = q[b].T
        t[E] = 1.0
        qTa[b] = t.reshape(KC, 128, N)
        t = np.zeros((CPAD, N), np.float32)
        t[:E] = kv[b].T
        t[E] = 1.0
        kvTa[b] = t.reshape(KC, 128, N)
        mf = np.where(mask[b] == 0, MASK_NEG, mask[b].astype(np.float32))
        mks[b] = np.ascontiguousarray(mf.reshape(NT, 128).T)

    in_maps = []
    for c in range(8):
        b, g = divmod(c, 4)
        hs = slice(DPC * g, DPC * (g + 1))

        wqa = np.zeros((CPAD, DPC), np.float32)
        wqa[:E] = Wq[hs].T * SCALE
        wqa[E] = bq[hs] * SCALE
        wka = np.zeros((CPAD, DPC), np.float32)
        wka[:E] = Wk[hs].T
        wka[E] = bk[hs]
        wva = np.zeros((CPAD, DPC), np.float32)
        wva[:E] = Wv[hs].T
        wva[E] = bv[hs]
        woT = np.ascontiguousarray(Wo[:, hs].T)  # [256, 1024]

        in_maps.append(
            {
                "qT": qTa[b],
                "kvT": kvTa[b],
                "wq": wqa.reshape(KC, 128, DPC),
                "wk": wka.reshape(KC, 128, DPC),
                "wv": wva.reshape(KC, 128, DPC),
                "wo": woT.reshape(2, 128, E),
                "mk": mks[b],
                "ones1": np.ones((128, NT, HPC, 1), np.float32),
            }
        )
    return in_maps


def kernel(q, kv, mask, Wq, bq, Wkv, bkv, Wo, bo, _repeat=1):
    from concourse.bass_utils import run_bass_kernel_spmd

    key = f"nc_{_repeat}"
    if key not in _CACHED:
        _CACHED[key] = build_bass(repeat=_repeat)
    nc = _CACHED[key]

    in_maps = make_in_maps(q, kv, mask, Wq, bq, Wkv, bkv, Wo, bo)
    res = run_bass_kernel_spmd(nc, in_maps, core_ids=list(range(8)))
    _CACHED["last_result"] = res

    bo = np.asarray(bo, dtype=np.float32)
    outs = [res.results[c]["outp"] for c in range(8)]
    out = np.stack(
        [
            outs[0] + outs[1] + outs[2] + outs[3],
            outs[4] + outs[5] + outs[6] + outs[7],
        ]
    )
    out += bo[None, None, :]
    return out.astype(np.float32)



# revision 2
# speedup vs baseline: 1.0827x; 1.0827x over previous
"""Trainium2 Bass kernel for nn_MultiHeadAttention_58548994179754 (v2, fp16).

Sharding: 8 cores = 2 batches x 4 head-groups (4 heads x 64 dims per core).

v2 design vs baseline:
  - fp16 datapath end to end (inputs, weights, xq/xk/xv, u=exp(logits), attn):
    halves DMA + SBUF traffic; PE rate identical (1 cycle/row for fp16 at any
    free size, same as f32r at >=256).
  - Q/K projections contract over 8x128 (no bias row); biases are applied by
    the PSUM->SBUF copy (tensor_scalar add with a per-partition bias AP).
    V keeps the 9th (bias) chunk and carries the softmax-denominator ones
    column inside the augmented V weights.
  - exp split across engines: 80/128 tiles on ACT (true exp, fp16 out); 48
    on DVE via the Schraudolph trick (y = x*1477.32 + 15300.7 -> i16, bitcast
    fp16 == exp(x) to ~3%), so the combined exp rate outpaces PE and the
    software-pipelined key-tile loop (QK/exp of kt+1 emitted before AV of kt)
    never stalls the tensor engine (p-state stays at full clock).
  - attention accumulators spill RAW (pre-normalize) to SBUF fp16 right after
    the last AV matmul, freeing their PSUM banks ~3 us earlier; the softmax
    normalize (recip + partition-broadcast + mul) runs lazily on DVE/Pool
    during the next group's key-tile loop, off the PE critical path.
  - PE-filler blocks at every attention-group boundary (Q-half1 projection
    split over two boundaries, prev-qb output projection at the third, last
    qb's at the tail) keep the PE warm while accumulators spill/normalize.
  - DMAs are consolidated (weights 1 descriptor each, q/kv as chunk-pairs,
    outputs paired) because each descriptor costs ~630ns on the shared HWDGE;
    queues split SP: q+out, ACT: kv.
Host: out[b] = sum of the 4 cores' fp16->f32 partials + bo.
"""

import os
import sys

import numpy as np

sys.path.insert(0, "/opt/trn_rl_repo")

B, N, E = 2, 2048, 1024
H, D = 16, 64
HPC = 4  # heads per core
DPC = HPC * D  # 256 output dims per core
SCALE = float(E) ** -0.5
KCQ = 8  # contraction chunks for Q/K (no bias row)
KCV = 9  # contraction chunks for V (8x128 data + bias/ones row)
NT = N // 128  # 16 key tiles
W65 = HPC * (D + 1)  # V output width: 4 heads x (64 dims + ones col)
MASK_NEG = np.float32(-60.0)  # masked logit offset (fp16-safe; exp -> 0)
S16 = np.float32(1024.0 / np.log(2.0))  # Schraudolph fp16 scale
B16 = np.float32(15.0 * 1024.0 - 59.3)  # fp16 exponent bias - minimax shift
_CACHED = {}


def build_bass(repeat=1):
    import concourse.bass as bass
    import concourse.mybir as mybir
    import concourse.tile as tile
    from concourse import bacc

    f32 = mybir.dt.float32
    f16 = mybir.dt.float16
    i16 = mybir.dt.int16
    Exp = mybir.ActivationFunctionType.Exp
    Copy = mybir.ActivationFunctionType.Copy
    Alu = mybir.AluOpType

    nc = bacc.Bacc("TRN2", target_bir_lowering=False)

    qT = nc.dram_tensor("qT", (KCQ, 128, N), f16, kind="ExternalInput")
    kvT = nc.dram_tensor("kvT", (KCV, 128, N), f16, kind="ExternalInput")
    wq = nc.dram_tensor("wq", (KCQ, 128, DPC), f16, kind="ExternalInput")
    wk = nc.dram_tensor("wk", (KCQ, 128, DPC), f16, kind="ExternalInput")
    wv = nc.dram_tensor("wv", (KCV, 128, W65), f16, kind="ExternalInput")
    wo = nc.dram_tensor("wo", (2, 128, E), f16, kind="ExternalInput")
    mk = nc.dram_tensor("mk", (128, NT), f32, kind="ExternalInput")
    mk16 = nc.dram_tensor("mk16", (128, NT), f32, kind="ExternalInput")
    bqk = nc.dram_tensor("bqk", (128, 4), f32, kind="ExternalInput")
    outp = nc.dram_tensor("outp", (NT, 128, E), f16, kind="ExternalOutput")

    with tile.TileContext(nc) as tc:
        with (
            tc.tile_pool(name="const", bufs=1) as const,
            tc.tile_pool(name="io", bufs=2) as io_pool,
            tc.tile_pool(name="ups", bufs=4) as upool,
            tc.tile_pool(name="rps", bufs=2) as rpool,
            tc.tile_pool(name="ps", bufs=2, space="PSUM") as ps,
            tc.tile_pool(name="avp", bufs=2, space="PSUM") as avp,
        ):
            wq_sb = const.tile([128, KCQ, DPC], f16, name="wq_sb")
            wk_sb = const.tile([128, KCQ, DPC], f16, name="wk_sb")
            wv_sb = const.tile([128, KCV, W65], f16, name="wv_sb")
            wo_sb = const.tile([128, 2, E], f16, name="wo_sb")
            mk_sb = const.tile([128, NT], f32, name="mk_sb")
            mk16_sb = const.tile([128, NT], f32, name="mk16_sb")
            bqk_sb = const.tile([128, 4], f32, name="bqk_sb")
            qTp = [const.tile([128, 2, N], f16, name=f"qTp{p}") for p in range(4)]
            kvTp = [const.tile([128, 2, N], f16, name=f"kvTp{p}") for p in range(5)]

            def qTs(kc):
                return qTp[kc // 2][:, kc % 2, :]

            def kvTs(kc):
                return kvTp[kc // 2][:, kc % 2, :]
            xqT = const.tile([128, 2, N], f16, name="xqT")
            xkT = const.tile([128, 2, N], f16, name="xkT")
            xv = const.tile([128, NT, W65], f16, name="xv")
            attn = const.tile([128, 2, N], f16, name="attn")
            # raw (pre-normalize) attention accumulators: [qb][j=2*hp+h2]
            avraw = const.tile([65, 2, 4, 1024], f16, name="avraw")
            warm = const.tile([1, 8], f32, name="warm")

            # loop-invariant loads (outside the repeat body), split over the
            # two HWDGE queues; early-needed weights (wq/wk) lead the SP
            # queue so the body's qT/kvT streams aren't stuck behind them
            nc.sync.dma_start(wq_sb[:], wq[:].rearrange("k p d -> p k d"))
            nc.sync.dma_start(bqk_sb[:], bqk[:])
            nc.scalar.dma_start(wk_sb[:], wk[:].rearrange("k p d -> p k d"))
            nc.scalar.dma_start(wv_sb[:], wv[:].rearrange("k p d -> p k d"))
            nc.scalar.dma_start(wo_sb[:], wo[:].rearrange("k p d -> p k d"))
            nc.scalar.dma_start(mk_sb[:], mk[:])
            nc.scalar.dma_start(mk16_sb[:], mk16[:])
            # warm the ACT exp table early so its load overlaps phase A
            nc.vector.memset(warm[:], 0.0)
            nc.scalar.activation(warm[:], warm[:], Exp)

            def body(_iv=None):
                # -------- input DMA: q on the SP queue, kv on the ACT queue
                for p in range(4):
                    nc.sync.dma_start(
                        qTp[p][:], qT[2 * p : 2 * p + 2].rearrange("k p n -> p k n")
                    )
                for p in range(4):
                    nc.scalar.dma_start(
                        kvTp[p][:], kvT[2 * p : 2 * p + 2].rearrange("k p n -> p k n")
                    )
                nc.scalar.dma_start(kvTp[4][:, 0, :], kvT[8])

                # -------- phase A: projections --------
                # Q: xqT[m, q] = (Wq_m^T q)*SCALE + bq_m*SCALE
                # (half 0 here; half 1 is emitted later as PE filler between
                # attention groups)
                def emit_qproj(half, m):
                    qp = ps.tile([128, 1024], f32, tag="ps", name=f"qp{half}{m}")
                    for kc in range(KCQ):
                        for s in range(2):
                            nc.tensor.matmul(
                                qp[:, s * 512 : (s + 1) * 512],
                                wq_sb[:, kc, m * 128 : (m + 1) * 128],
                                qTs(kc)[:, half * 1024 + s * 512 : half * 1024 + (s + 1) * 512],
                                start=(kc == 0),
                                stop=(kc == KCQ - 1),
                            )
                    nc.vector.tensor_scalar(
                        xqT[:, m, half * 1024 : (half + 1) * 1024],
                        qp[:],
                        bqk_sb[:, m : m + 1],
                        None,
                        Alu.add,
                    )

                # Q half0 (ps pool) interleaved per-chunk with K half0 (avp
                # pool, idle during phase A): PE consumes each input chunk as
                # it lands instead of idling through the DMA head
                qps = [ps.tile([128, 1024], f32, tag="ps", name=f"qp0{m}") for m in range(2)]
                kps = [avp.tile([128, 1024], f32, tag="av", name=f"kp0{m}") for m in range(2)]
                for kc in range(KCQ):
                    for m in range(2):
                        for s in range(2):
                            nc.tensor.matmul(
                                qps[m][:, s * 512 : (s + 1) * 512],
                                wq_sb[:, kc, m * 128 : (m + 1) * 128],
                                qTs(kc)[:, s * 512 : (s + 1) * 512],
                                start=(kc == 0),
                                stop=(kc == KCQ - 1),
                            )
                    for m in range(2):
                        for s in range(2):
                            nc.tensor.matmul(
                                kps[m][:, s * 512 : (s + 1) * 512],
                                wk_sb[:, kc, m * 128 : (m + 1) * 128],
                                kvTs(kc)[:, s * 512 : (s + 1) * 512],
                                start=(kc == 0),
                                stop=(kc == KCQ - 1),
                            )
                # xk copies first: they gate the K-half1 psum ring slots
                for m in range(2):
                    nc.vector.tensor_scalar(
                        xkT[:, m, 0:1024], kps[m][:], bqk_sb[:, 2 + m : 3 + m], None, Alu.add
                    )
                for m in range(2):
                    nc.vector.tensor_scalar(
                        xqT[:, m, 0:1024], qps[m][:], bqk_sb[:, m : m + 1], None, Alu.add
                    )
                # K half1 (avp ring again)
                kps1 = [avp.tile([128, 1024], f32, tag="av", name=f"kp1{m}") for m in range(2)]
                for kc in range(KCQ):
                    for m in range(2):
                        for s in range(2):
                            nc.tensor.matmul(
                                kps1[m][:, s * 512 : (s + 1) * 512],
                                wk_sb[:, kc, m * 128 : (m + 1) * 128],
                                kvTs(kc)[:, 1024 + s * 512 : 1024 + (s + 1) * 512],
                                start=(kc == 0),
                                stop=(kc == KCQ - 1),
                            )
                for m in range(2):
                    nc.vector.tensor_scalar(
                        xkT[:, m, 1024:2048], kps1[m][:], bqk_sb[:, 2 + m : 3 + m], None, Alu.add
                    )
                # V (natural layout, 65th column per head = ones for denom)
                for nt in range(NT):
                    vp = ps.tile([128, 1024], f32, tag="ps", name=f"vp{nt}")
                    for kc in range(KCV):
                        nc.tensor.matmul(
                            vp[:, 0:W65],
                            kvTs(kc)[:, nt * 128 : (nt + 1) * 128],
                            wv_sb[:, kc, :],
                            start=(kc == 0),
                            stop=(kc == KCV - 1),
                        )
                    nc.scalar.activation(xv[:, nt, :], vp[:, 0:W65], Copy)

                # -------- phase B: attention (+ interleaved C of prev qb) ----
                c_state = {}

                def emit_c(nt):
                    op = ps.tile([128, 1024], f32, tag="ps", name=f"op{nt}")
                    for c in range(2):
                        for s in range(2):
                            nc.tensor.matmul(
                                op[:, s * 512 : (s + 1) * 512],
                                attn[:, c, nt * 128 : (nt + 1) * 128],
                                wo_sb[:, c, s * 512 : (s + 1) * 512],
                                start=(c == 0),
                                stop=(c == 1),
                            )
                    if nt % 2 == 0:
                        c_state["ot"] = io_pool.tile(
                            [128, 2, 1024], f16, tag="ot", name=f"ot{nt}"
                        )
                    ot = c_state["ot"]
                    # PSUM->SBUF copies round-robin ACT/Pool: both are idle
                    # during C blocks, and DVE must stay clear for the next
                    # group's exp stream (engine queues run in order)
                    if nt % 2 == 0:
                        nc.scalar.activation(ot[:, 0, :], op[:], Copy)
                    else:
                        nc.vector.tensor_copy(ot[:, 1, :], op[:])
                        nc.sync.dma_start(
                            outp[nt - 1 : nt + 1].rearrange("j p e -> p j e"), ot[:]
                        )

                def emit_norm(qb, hp, h2):
                    # lazy softmax normalize from the raw SBUF accumulators
                    j = 2 * hp + h2
                    po = 64 * h2
                    rs = rpool.tile([1, 1024], f16, tag="rs", name="rs")
                    nc.vector.tensor_scalar(
                        rs[:], avraw[64:65, qb, j, :], 1.0, None, Alu.add
                    )
                    with nc.allow_low_precision(reason="softmax denom recip in fp16"):
                        nc.vector.reciprocal(rs[:], rs[:])
                    rb = rpool.tile([64, 1024], f16, tag="rb", name="rb")
                    nc.gpsimd.partition_broadcast(rb[:], rs[0:1, :])
                    # all-fp16 operands -> DVE 2x mode
                    nc.vector.tensor_mul(
                        attn[po : po + 64, hp, qb * 1024 : (qb + 1) * 1024],
                        avraw[0:64, qb, j, :],
                        rb[:],
                    )

                def emit_qk(qb, hp, kt):
                    qks = []
                    for h2 in range(2):
                        po = 64 * h2
                        qk = ps.tile([128, 1024], f32, tag="ps", name=f"qk{h2}")
                        for s in range(2):
                            nc.tensor.matmul(
                                qk[:, s * 512 : (s + 1) * 512],
                                xkT[po : po + 64, hp, kt * 128 : (kt + 1) * 128],
                                xqT[
                                    po : po + 64,
                                    hp,
                                    qb * 1024 + s * 512 : qb * 1024 + (s + 1) * 512,
                                ],
                                start=True,
                                stop=True,
                            )
                        qks.append(qk)
                    return qks

                def emit_exp(kt, h2, qk):
                    u = upool.tile([128, 1024], f16, tag="u", name=f"u{h2}")
                    if h2 == 1 and kt % 4 != 1:
                        # Schraudolph: (qk*S16 + mask16) -> i16, bitcast fp16
                        # == exp(qk + mask)
                        nc.vector.tensor_scalar(
                            u[:].bitcast(i16),
                            qk[:],
                            float(S16),
                            mk16_sb[:, kt : kt + 1],
                            Alu.mult,
                            Alu.add,
                        )
                    else:
                        nc.scalar.activation(
                            u[:], qk[:], Exp, bias=mk_sb[:, kt : kt + 1], scale=1.0
                        )
                    return u

                # group schedule: (qb, hp) plus PE-filler ops emitted at the
                # top of each group (before its kt loop, after its prologue
                # QK) so PE never idles while the previous group's PSUM
                # accumulators spill / normalize
                groups = [(0, 0), (0, 1), (1, 0), (1, 1)]
                fillers = {
                    1: lambda: emit_qproj(1, 0),
                    2: lambda: emit_qproj(1, 1),
                    3: lambda: [emit_c(nt) for nt in range(0, 8)],
                }
                prev = None
                for gi, (qb, hp) in enumerate(groups):
                    avs = [
                        avp.tile([128, 1024], f32, tag="av", name=f"av{qb}{hp}{i}")
                        for i in range(2)
                    ]
                    # prologue: QK + exp for kt0 emitted before fillers
                    # so filler copies (ACT/DVE) queue behind them
                    qk_cur = emit_qk(qb, hp, 0)
                    u_cur = [emit_exp(0, h2, qk_cur[h2]) for h2 in range(2)]
                    if gi in fillers:
                        fillers[gi]()
                    for kt in range(NT):
                        # next kt's QK+exp first: PE never waits on exp
                        if kt + 1 < NT:
                            qk_next = emit_qk(qb, hp, kt + 1)
                            u_next = [emit_exp(kt + 1, h2, qk_next[h2]) for h2 in range(2)]
                        else:
                            u_next = None
                        # previous group's lazy normalize, spread thin
                        if prev is not None and kt in (2, 5):
                            emit_norm(prev[0], prev[1], 0 if kt == 2 else 1)
                        for h2 in range(2):
                            h = 2 * hp + h2
                            for s in range(2):
                                nc.tensor.matmul(
                                    avs[h2][0:65, s * 512 : (s + 1) * 512],
                                    xv[:, kt, h * 65 : (h + 1) * 65],
                                    u_cur[h2][:, s * 512 : (s + 1) * 512],
                                    start=(kt == 0),
                                    stop=(kt == NT - 1),
                                )
                        u_cur = u_next
                    # spill raw accumulators to SBUF: frees PSUM banks fast
                    # so the next group's accumulators alloc without waiting
                    # on the (slow) normalize chain
                    nc.vector.tensor_copy(avraw[:, qb, 2 * hp, :], avs[0][0:65, :])
                    nc.scalar.activation(
                        avraw[:, qb, 2 * hp + 1, :], avs[1][0:65, :], Copy
                    )
                    prev = (qb, hp)
                # tail: last group's normalize + last qb's output projection
                for h2 in range(2):
                    emit_norm(1, 1, h2)
                for nt in range(8, 16):
                    emit_c(nt)

            if repeat == 1:
                body()
            else:
                with tc.For_i(0, repeat, 1) as _i:
                    body(_i)

    nc.compile()
    return nc


def make_in_maps(q, kv, mask, Wq, bq, Wkv, bkv, Wo, bo):
    q = np.asarray(q, dtype=np.float32)
    kv = np.asarray(kv, dtype=np.float32)
    mask = np.asarray(mask)
    Wq = np.asarray(Wq, dtype=np.float32)
    bq = np.asarray(bq, dtype=np.float32)
    Wkv = np.asarray(Wkv, dtype=np.float32)
    bkv = np.asarray(bkv, dtype=np.float32)
    Wo = np.asarray(Wo, dtype=np.float32)

    Wk, Wv = Wkv[:E], Wkv[E:]
    bk, bv = bkv[:E], bkv[E:]

    qTa, kvTa, mks, mk16s = {}, {}, {}, {}
    for b in range(B):
        qTa[b] = np.ascontiguousarray(q[b].T).astype(np.float16).reshape(KCQ, 128, N)
        t = np.zeros((KCV * 128, N), np.float16)
        t[:E] = kv[b].T
        t[E] = 1.0
        kvTa[b] = t.reshape(KCV, 128, N)
        mf = np.where(mask[b] == 0, MASK_NEG, mask[b].astype(np.float32))
        mks[b] = np.ascontiguousarray(mf.reshape(NT, 128).T)
        mk16s[b] = np.ascontiguousarray((mf * S16 + B16).reshape(NT, 128).T)

    in_maps = []
    for c in range(8):
        b, g = divmod(c, 4)
        hs = slice(DPC * g, DPC * (g + 1))

        wqa = (Wq[hs].T * SCALE).astype(np.float16).reshape(KCQ, 128, DPC)
        wka = Wk[hs].T.astype(np.float16).reshape(KCQ, 128, DPC)
        # V weights augmented with bias row and per-head ones column
        wva = np.zeros((KCV * 128, W65), np.float16)
        Wv_core = Wv[hs]  # [256, 1024]
        for h in range(HPC):
            wva[:E, h * 65 : h * 65 + 64] = Wv_core[h * 64 : (h + 1) * 64].T
            wva[E, h * 65 : h * 65 + 64] = bv[hs][h * 64 : (h + 1) * 64]
            wva[E, h * 65 + 64] = 1.0
        woT = np.ascontiguousarray(Wo[:, hs].T).astype(np.float16)  # [256, 1024]
        bqka = np.zeros((128, 4), np.float32)
        bqka[:, 0] = bq[hs][0:128] * SCALE
        bqka[:, 1] = bq[hs][128:256] * SCALE
        bqka[:, 2] = bk[hs][0:128]
        bqka[:, 3] = bk[hs][128:256]

        in_maps.append(
            {
                "qT": qTa[b],
                "kvT": kvTa[b],
                "wq": wqa,
                "wk": wka,
                "wv": wva.reshape(KCV, 128, W65),
                "wo": woT.reshape(2, 128, E),
                "mk": mks[b],
                "mk16": mk16s[b],
                "bqk": bqka,
            }
        )
    return in_maps


def kernel(q, kv, mask, Wq, bq, Wkv, bkv, Wo, bo, _repeat=1):
    from concourse.bass_utils import run_bass_kernel_spmd

    key = f"nc_{_repeat}"
    if key not in _CACHED:
        _CACHED[key] = build_bass(repeat=_repeat)
    nc = _CACHED[key]

    in_maps = make_in_maps(q, kv, mask, Wq, bq, Wkv, bkv, Wo, bo)
    res = run_bass_kernel_spmd(nc, in_maps, core_ids=list(range(8)))
    _CACHED["last_result"] = res

    bo = np.asarray(bo, dtype=np.float32)
    outs = [np.asarray(res.results[c]["outp"], np.float32).reshape(N, E) for c in range(8)]
    out = np.stack(
        [
            outs[0] + outs[1] + outs[2] + outs[3],
            outs[4] + outs[5] + outs[6] + outs[7],
        ]
    )
    out += bo[None, None, :]
    return out.astype(np.float32)


# revision 3
# speedup vs baseline: 1.2062x; 1.1141x over previous
"""Trainium2 Bass kernel for nn_MultiHeadAttention_58548994179754 (v2, fp16).

Sharding: 8 cores = 2 batches x 4 head-groups (4 heads x 64 dims per core).

v2 design vs baseline:
  - fp16 datapath end to end (inputs, weights, xq/xk/xv, u=exp(logits), attn):
    halves DMA + SBUF traffic; PE rate identical (1 cycle/row for fp16 at any
    free size, same as f32r at >=256).
  - Q/K projections contract over 8x128 (no bias row); biases are applied by
    the PSUM->SBUF copy (tensor_scalar add with a per-partition bias AP).
    V keeps the 9th (bias) chunk and carries the softmax-denominator ones
    column inside the augmented V weights.
  - exp split 64/64 across ACT (true exp, fp16 out) and DVE (Schraudolph:
    y = x*1477.32 + 15300.7 -> i16, bitcast fp16 == exp(x) to ~3%). HW A/B
    showed ACT exp costs ~1.5us/tile on silicon (vs 1.04 modeled): all-ACT
    measured 432us, 80/48 split 360-383us, 64/64 split 332us. The
    software-pipelined key-tile loop (QK/exp of kt+1 emitted before AV of kt)
    keeps the tensor engine at full p-state clock.
  - attention accumulators spill RAW (pre-normalize) to SBUF fp16 right after
    the last AV matmul, freeing their PSUM banks ~3 us earlier; the softmax
    normalize (recip + partition-broadcast + mul) runs lazily on DVE/Pool
    during the next group's key-tile loop, off the PE critical path.
  - PE-filler blocks at every attention-group boundary (Q-half1 projection
    split over two boundaries, prev-qb output projection at the third, last
    qb's at the tail) keep the PE warm while accumulators spill/normalize.
  - DMAs are consolidated (weights 1 descriptor each, q/kv as chunk-pairs,
    outputs paired) because each descriptor costs ~630ns on the shared HWDGE;
    queues split SP: q+out, ACT: kv.
Host: out[b] = sum of the 4 cores' fp16->f32 partials + bo.
"""

import os
import sys

import numpy as np

sys.path.insert(0, "/opt/trn_rl_repo")

B, N, E = 2, 2048, 1024
H, D = 16, 64
HPC = 4  # heads per core
DPC = HPC * D  # 256 output dims per core
SCALE = float(E) ** -0.5
KCQ = 8  # contraction chunks for Q/K (no bias row)
KCV = 9  # contraction chunks for V (8x128 data + bias/ones row)
NT = N // 128  # 16 key tiles
W65 = HPC * (D + 1)  # V output width: 4 heads x (64 dims + ones col)
MASK_NEG = np.float32(-60.0)  # masked logit offset (fp16-safe; exp -> 0)
S16 = np.float32(1024.0 / np.log(2.0))  # Schraudolph fp16 scale
B16 = np.float32(15.0 * 1024.0 - 59.3)  # fp16 exponent bias - minimax shift
_CACHED = {}


def build_bass(repeat=1):
    import concourse.bass as bass
    import concourse.mybir as mybir
    import concourse.tile as tile
    from concourse import bacc

    f32 = mybir.dt.float32
    f16 = mybir.dt.float16
    i16 = mybir.dt.int16
    Exp = mybir.ActivationFunctionType.Exp
    Copy = mybir.ActivationFunctionType.Copy
    Alu = mybir.AluOpType

    nc = bacc.Bacc("TRN2", target_bir_lowering=False)

    qT = nc.dram_tensor("qT", (KCQ, 128, N), f16, kind="ExternalInput")
    kvT = nc.dram_tensor("kvT", (KCV, 128, N), f16, kind="ExternalInput")
    wq = nc.dram_tensor("wq", (KCQ, 128, DPC), f16, kind="ExternalInput")
    wk = nc.dram_tensor("wk", (KCQ, 128, DPC), f16, kind="ExternalInput")
    wv = nc.dram_tensor("wv", (KCV, 128, W65), f16, kind="ExternalInput")
    wo = nc.dram_tensor("wo", (2, 128, E), f16, kind="ExternalInput")
    mk = nc.dram_tensor("mk", (128, NT), f32, kind="ExternalInput")
    mk16 = nc.dram_tensor("mk16", (128, NT), f32, kind="ExternalInput")
    bqk = nc.dram_tensor("bqk", (128, 4), f32, kind="ExternalInput")
    outp = nc.dram_tensor("outp", (NT, 128, E), f16, kind="ExternalOutput")

    with tile.TileContext(nc) as tc:
        with (
            tc.tile_pool(name="const", bufs=1) as const,
            tc.tile_pool(name="io", bufs=2) as io_pool,
            tc.tile_pool(name="ups", bufs=4) as upool,
            tc.tile_pool(name="rps", bufs=2) as rpool,
            tc.tile_pool(name="ps", bufs=2, space="PSUM") as ps,
            tc.tile_pool(name="avp", bufs=2, space="PSUM") as avp,
        ):
            wq_sb = const.tile([128, KCQ, DPC], f16, name="wq_sb")
            wk_sb = const.tile([128, KCQ, DPC], f16, name="wk_sb")
            wv_sb = const.tile([128, KCV, W65], f16, name="wv_sb")
            wo_sb = const.tile([128, 2, E], f16, name="wo_sb")
            mk_sb = const.tile([128, NT], f32, name="mk_sb")
            mk16_sb = const.tile([128, NT], f32, name="mk16_sb")
            bqk_sb = const.tile([128, 4], f32, name="bqk_sb")
            qTp = [const.tile([128, 2, N], f16, name=f"qTp{p}") for p in range(4)]
            kvTp = [const.tile([128, 2, N], f16, name=f"kvTp{p}") for p in range(5)]

            def qTs(kc):
                return qTp[kc // 2][:, kc % 2, :]

            def kvTs(kc):
                return kvTp[kc // 2][:, kc % 2, :]
            xqT = const.tile([128, 2, N], f16, name="xqT")
            xkT = const.tile([128, 2, N], f16, name="xkT")
            xv = const.tile([128, NT, W65], f16, name="xv")
            attn = const.tile([128, 2, N], f16, name="attn")
            # raw (pre-normalize) attention accumulators: [qb][j=2*hp+h2]
            avraw = const.tile([65, 2, 4, 1024], f16, name="avraw")
            warm = const.tile([1, 8], f32, name="warm")

            # loop-invariant loads (outside the repeat body), split over the
            # two HWDGE queues; early-needed weights (wq/wk) lead the SP
            # queue so the body's qT/kvT streams aren't stuck behind them
            nc.sync.dma_start(wq_sb[:], wq[:].rearrange("k p d -> p k d"))
            nc.sync.dma_start(bqk_sb[:], bqk[:])
            nc.scalar.dma_start(wk_sb[:], wk[:].rearrange("k p d -> p k d"))
            nc.scalar.dma_start(wv_sb[:], wv[:].rearrange("k p d -> p k d"))
            nc.scalar.dma_start(wo_sb[:], wo[:].rearrange("k p d -> p k d"))
            nc.scalar.dma_start(mk_sb[:], mk[:])
            nc.scalar.dma_start(mk16_sb[:], mk16[:])
            # warm the ACT exp table early so its load overlaps phase A
            nc.vector.memset(warm[:], 0.0)
            nc.scalar.activation(warm[:], warm[:], Exp)

            def body(_iv=None):
                # -------- input DMA: q on the SP queue, kv on the ACT queue
                for p in range(4):
                    nc.sync.dma_start(
                        qTp[p][:], qT[2 * p : 2 * p + 2].rearrange("k p n -> p k n")
                    )
                for p in range(4):
                    nc.scalar.dma_start(
                        kvTp[p][:], kvT[2 * p : 2 * p + 2].rearrange("k p n -> p k n")
                    )
                nc.scalar.dma_start(kvTp[4][:, 0, :], kvT[8])

                # -------- phase A: projections --------
                # Q: xqT[m, q] = (Wq_m^T q)*SCALE + bq_m*SCALE
                # (half 0 here; half 1 is emitted later as PE filler between
                # attention groups)
                def emit_qproj(half, m):
                    qp = ps.tile([128, 1024], f32, tag="ps", name=f"qp{half}{m}")
                    for kc in range(KCQ):
                        for s in range(2):
                            nc.tensor.matmul(
                                qp[:, s * 512 : (s + 1) * 512],
                                wq_sb[:, kc, m * 128 : (m + 1) * 128],
                                qTs(kc)[:, half * 1024 + s * 512 : half * 1024 + (s + 1) * 512],
                                start=(kc == 0),
                                stop=(kc == KCQ - 1),
                            )
                    nc.vector.tensor_scalar(
                        xqT[:, m, half * 1024 : (half + 1) * 1024],
                        qp[:],
                        bqk_sb[:, m : m + 1],
                        None,
                        Alu.add,
                    )

                # Q half0 (ps pool) interleaved per-chunk with K half0 (avp
                # pool, idle during phase A): PE consumes each input chunk as
                # it lands instead of idling through the DMA head
                qps = [ps.tile([128, 1024], f32, tag="ps", name=f"qp0{m}") for m in range(2)]
                kps = [avp.tile([128, 1024], f32, tag="av", name=f"kp0{m}") for m in range(2)]
                for kc in range(KCQ):
                    for m in range(2):
                        for s in range(2):
                            nc.tensor.matmul(
                                qps[m][:, s * 512 : (s + 1) * 512],
                                wq_sb[:, kc, m * 128 : (m + 1) * 128],
                                qTs(kc)[:, s * 512 : (s + 1) * 512],
                                start=(kc == 0),
                                stop=(kc == KCQ - 1),
                            )
                    for m in range(2):
                        for s in range(2):
                            nc.tensor.matmul(
                                kps[m][:, s * 512 : (s + 1) * 512],
                                wk_sb[:, kc, m * 128 : (m + 1) * 128],
                                kvTs(kc)[:, s * 512 : (s + 1) * 512],
                                start=(kc == 0),
                                stop=(kc == KCQ - 1),
                            )
                # xk copies first: they gate the K-half1 psum ring slots
                for m in range(2):
                    nc.vector.tensor_scalar(
                        xkT[:, m, 0:1024], kps[m][:], bqk_sb[:, 2 + m : 3 + m], None, Alu.add
                    )
                for m in range(2):
                    nc.vector.tensor_scalar(
                        xqT[:, m, 0:1024], qps[m][:], bqk_sb[:, m : m + 1], None, Alu.add
                    )
                # K half1 (avp ring again)
                kps1 = [avp.tile([128, 1024], f32, tag="av", name=f"kp1{m}") for m in range(2)]
                for kc in range(KCQ):
                    for m in range(2):
                        for s in range(2):
                            nc.tensor.matmul(
                                kps1[m][:, s * 512 : (s + 1) * 512],
                                wk_sb[:, kc, m * 128 : (m + 1) * 128],
                                kvTs(kc)[:, 1024 + s * 512 : 1024 + (s + 1) * 512],
                                start=(kc == 0),
                                stop=(kc == KCQ - 1),
                            )
                for m in range(2):
                    nc.vector.tensor_scalar(
                        xkT[:, m, 1024:2048], kps1[m][:], bqk_sb[:, 2 + m : 3 + m], None, Alu.add
                    )
                # V (natural layout, 65th column per head = ones for denom)
                for nt in range(NT):
                    vp = ps.tile([128, 1024], f32, tag="ps", name=f"vp{nt}")
                    for kc in range(KCV):
                        nc.tensor.matmul(
                            vp[:, 0:W65],
                            kvTs(kc)[:, nt * 128 : (nt + 1) * 128],
                            wv_sb[:, kc, :],
                            start=(kc == 0),
                            stop=(kc == KCV - 1),
                        )
                    nc.scalar.activation(xv[:, nt, :], vp[:, 0:W65], Copy)

                # -------- phase B: attention (+ interleaved C of prev qb) ----
                c_state = {}

                def emit_c(nt):
                    op = ps.tile([128, 1024], f32, tag="ps", name=f"op{nt}")
                    for c in range(2):
                        for s in range(2):
                            nc.tensor.matmul(
                                op[:, s * 512 : (s + 1) * 512],
                                attn[:, c, nt * 128 : (nt + 1) * 128],
                                wo_sb[:, c, s * 512 : (s + 1) * 512],
                                start=(c == 0),
                                stop=(c == 1),
                            )
                    if nt % 2 == 0:
                        c_state["ot"] = io_pool.tile(
                            [128, 2, 1024], f16, tag="ot", name=f"ot{nt}"
                        )
                    ot = c_state["ot"]
                    # PSUM->SBUF copies round-robin ACT/Pool: both are idle
                    # during C blocks, and DVE must stay clear for the next
                    # group's exp stream (engine queues run in order)
                    if nt % 2 == 0:
                        nc.scalar.activation(ot[:, 0, :], op[:], Copy)
                    else:
                        nc.vector.tensor_copy(ot[:, 1, :], op[:])
                        nc.sync.dma_start(
                            outp[nt - 1 : nt + 1].rearrange("j p e -> p j e"), ot[:]
                        )

                def emit_norm(qb, hp, h2):
                    # lazy softmax normalize from the raw SBUF accumulators
                    j = 2 * hp + h2
                    po = 64 * h2
                    rs = rpool.tile([1, 1024], f16, tag="rs", name="rs")
                    nc.vector.tensor_scalar(
                        rs[:], avraw[64:65, qb, j, :], 1.0, None, Alu.add
                    )
                    with nc.allow_low_precision(reason="softmax denom recip in fp16"):
                        nc.vector.reciprocal(rs[:], rs[:])
                    rb = rpool.tile([64, 1024], f16, tag="rb", name="rb")
                    nc.gpsimd.partition_broadcast(rb[:], rs[0:1, :])
                    # all-fp16 operands -> DVE 2x mode
                    nc.vector.tensor_mul(
                        attn[po : po + 64, hp, qb * 1024 : (qb + 1) * 1024],
                        avraw[0:64, qb, j, :],
                        rb[:],
                    )

                def emit_qk(qb, hp, kt):
                    qks = []
                    for h2 in range(2):
                        po = 64 * h2
                        qk = ps.tile([128, 1024], f32, tag="ps", name=f"qk{h2}")
                        for s in range(2):
                            nc.tensor.matmul(
                                qk[:, s * 512 : (s + 1) * 512],
                                xkT[po : po + 64, hp, kt * 128 : (kt + 1) * 128],
                                xqT[
                                    po : po + 64,
                                    hp,
                                    qb * 1024 + s * 512 : qb * 1024 + (s + 1) * 512,
                                ],
                                start=True,
                                stop=True,
                            )
                        qks.append(qk)
                    return qks

                def emit_exp(kt, h2, qk):
                    u = upool.tile([128, 1024], f16, tag="u", name=f"u{h2}")
                    if h2 == 1:
                        # Schraudolph: (qk*S16 + mask16) -> i16, bitcast fp16
                        # == exp(qk + mask)
                        nc.vector.tensor_scalar(
                            u[:].bitcast(i16),
                            qk[:],
                            float(S16),
                            mk16_sb[:, kt : kt + 1],
                            Alu.mult,
                            Alu.add,
                        )
                    else:
                        nc.scalar.activation(
                            u[:], qk[:], Exp, bias=mk_sb[:, kt : kt + 1], scale=1.0
                        )
                    return u

                # group schedule: (qb, hp) plus PE-filler ops emitted at the
                # top of each group (before its kt loop, after its prologue
                # QK) so PE never idles while the previous group's PSUM
                # accumulators spill / normalize
                groups = [(0, 0), (0, 1), (1, 0), (1, 1)]
                fillers = {
                    1: lambda: emit_qproj(1, 0),
                    2: lambda: emit_qproj(1, 1),
                    3: lambda: [emit_c(nt) for nt in range(0, 8)],
                }
                prev = None
                for gi, (qb, hp) in enumerate(groups):
                    avs = [
                        avp.tile([128, 1024], f32, tag="av", name=f"av{qb}{hp}{i}")
                        for i in range(2)
                    ]
                    # prologue: QK + exp for kt0 emitted before fillers
                    # so filler copies (ACT/DVE) queue behind them
                    qk_cur = emit_qk(qb, hp, 0)
                    u_cur = [emit_exp(0, h2, qk_cur[h2]) for h2 in range(2)]
                    if gi in fillers:
                        fillers[gi]()
                    for kt in range(NT):
                        # next kt's QK+exp first: PE never waits on exp
                        if kt + 1 < NT:
                            qk_next = emit_qk(qb, hp, kt + 1)
                            u_next = [emit_exp(kt + 1, h2, qk_next[h2]) for h2 in range(2)]
                        else:
                            u_next = None
                        # previous group's lazy normalize, spread thin
                        if prev is not None and kt in (2, 5):
                            emit_norm(prev[0], prev[1], 0 if kt == 2 else 1)
                        for h2 in range(2):
                            h = 2 * hp + h2
                            for s in range(2):
                                nc.tensor.matmul(
                                    avs[h2][0:65, s * 512 : (s + 1) * 512],
                                    xv[:, kt, h * 65 : (h + 1) * 65],
                                    u_cur[h2][:, s * 512 : (s + 1) * 512],
                                    start=(kt == 0),
                                    stop=(kt == NT - 1),
                                )
                        u_cur = u_next
                    # spill raw accumulators to SBUF: frees PSUM banks fast
                    # so the next group's accumulators alloc without waiting
                    # on the (slow) normalize chain
                    nc.vector.tensor_copy(avraw[:, qb, 2 * hp, :], avs[0][0:65, :])
                    nc.scalar.activation(
                        avraw[:, qb, 2 * hp + 1, :], avs[1][0:65, :], Copy
                    )
                    prev = (qb, hp)
                # tail: last group's normalize + last qb's output projection
                for h2 in range(2):
                    emit_norm(1, 1, h2)
                for nt in range(8, 16):
                    emit_c(nt)

            if repeat == 1:
                body()
            else:
                with tc.For_i(0, repeat, 1) as _i:
                    body(_i)

    nc.compile()
    return nc


def make_in_maps(q, kv, mask, Wq, bq, Wkv, bkv, Wo, bo):
    q = np.asarray(q, dtype=np.float32)
    kv = np.asarray(kv, dtype=np.float32)
    mask = np.asarray(mask)
    Wq = np.asarray(Wq, dtype=np.float32)
    bq = np.asarray(bq, dtype=np.float32)
    Wkv = np.asarray(Wkv, dtype=np.float32)
    bkv = np.asarray(bkv, dtype=np.float32)
    Wo = np.asarray(Wo, dtype=np.float32)

    Wk, Wv = Wkv[:E], Wkv[E:]
    bk, bv = bkv[:E], bkv[E:]

    qTa, kvTa, mks, mk16s = {}, {}, {}, {}
    for b in range(B):
        qTa[b] = np.ascontiguousarray(q[b].T).astype(np.float16).reshape(KCQ, 128, N)
        t = np.zeros((KCV * 128, N), np.float16)
        t[:E] = kv[b].T
        t[E] = 1.0
        kvTa[b] = t.reshape(KCV, 128, N)
        mf = np.where(mask[b] == 0, MASK_NEG, mask[b].astype(np.float32))
        mks[b] = np.ascontiguousarray(mf.reshape(NT, 128).T)
        mk16s[b] = np.ascontiguousarray((mf * S16 + B16).reshape(NT, 128).T)

    in_maps = []
    for c in range(8):
        b, g = divmod(c, 4)
        hs = slice(DPC * g, DPC * (g + 1))

        wqa = (Wq[hs].T * SCALE).astype(np.float16).reshape(KCQ, 128, DPC)
        wka = Wk[hs].T.astype(np.float16).reshape(KCQ, 128, DPC)
        # V weights augmented with bias row and per-head ones column
        wva = np.zeros((KCV * 128, W65), np.float16)
        Wv_core = Wv[hs]  # [256, 1024]
        for h in range(HPC):
            wva[:E, h * 65 : h * 65 + 64] = Wv_core[h * 64 : (h + 1) * 64].T
            wva[E, h * 65 : h * 65 + 64] = bv[hs][h * 64 : (h + 1) * 64]
            wva[E, h * 65 + 64] = 1.0
        woT = np.ascontiguousarray(Wo[:, hs].T).astype(np.float16)  # [256, 1024]
        bqka = np.zeros((128, 4), np.float32)
        bqka[:, 0] = bq[hs][0:128] * SCALE
        bqka[:, 1] = bq[hs][128:256] * SCALE
        bqka[:, 2] = bk[hs][0:128]
        bqka[:, 3] = bk[hs][128:256]

        in_maps.append(
            {
                "qT": qTa[b],
                "kvT": kvTa[b],
                "wq": wqa,
                "wk": wka,
                "wv": wva.reshape(KCV, 128, W65),
                "wo": woT.reshape(2, 128, E),
                "mk": mks[b],
                "mk16": mk16s[b],
                "bqk": bqka,
            }
        )
    return in_maps


def kernel(q, kv, mask, Wq, bq, Wkv, bkv, Wo, bo, _repeat=1):
    from concourse.bass_utils import run_bass_kernel_spmd

    key = f"nc_{_repeat}"
    if key not in _CACHED:
        _CACHED[key] = build_bass(repeat=_repeat)
    nc = _CACHED[key]

    in_maps = make_in_maps(q, kv, mask, Wq, bq, Wkv, bkv, Wo, bo)
    res = run_bass_kernel_spmd(nc, in_maps, core_ids=list(range(8)))
    _CACHED["last_result"] = res

    bo = np.asarray(bo, dtype=np.float32)
    outs = [np.asarray(res.results[c]["outp"], np.float32).reshape(N, E) for c in range(8)]
    out = np.stack(
        [
            outs[0] + outs[1] + outs[2] + outs[3],
            outs[4] + outs[5] + outs[6] + outs[7],
        ]
    )
    out += bo[None, None, :]
    return out.astype(np.float32)


# revision 4
# speedup vs baseline: 1.2338x; 1.0229x over previous
"""Trainium2 Bass kernel for nn_MultiHeadAttention_58548994179754 (v2, fp16).

Sharding: 8 cores = 2 batches x 4 head-groups (4 heads x 64 dims per core).

v2 design vs baseline:
  - fp16 datapath end to end (inputs, weights, xq/xk/xv, u=exp(logits), attn):
    halves DMA + SBUF traffic; PE rate identical (1 cycle/row for fp16 at any
    free size, same as f32r at >=256).
  - Q/K projections contract over 8x128 (no bias row); biases are applied by
    the PSUM->SBUF copy (tensor_scalar add with a per-partition bias AP).
    V keeps the 9th (bias) chunk and carries the softmax-denominator ones
    column inside the augmented V weights.
  - exp split 64/64 across ACT (true exp, fp16 out) and DVE (Schraudolph:
    y = x*1477.32 + 15300.7 -> i16, bitcast fp16 == exp(x) to ~3%). HW A/B
    showed ACT exp costs ~1.5us/tile on silicon (vs 1.04 modeled): all-ACT
    measured 432us, 80/48 split 360-383us, 64/64 split 332us. The
    software-pipelined key-tile loop (QK/exp of kt+1 emitted before AV of kt)
    keeps the tensor engine at full p-state clock.
  - attention accumulators spill RAW (pre-normalize) to SBUF fp16 right after
    the last AV matmul, freeing their PSUM banks ~3 us earlier; the softmax
    normalize (recip + partition-broadcast + mul) runs lazily on DVE/Pool
    during the next group's key-tile loop, off the PE critical path.
  - PE-filler blocks at every attention-group boundary (Q-half1 projection
    split over two boundaries, prev-qb output projection at the third, last
    qb's at the tail) keep the PE warm while accumulators spill/normalize.
  - DMAs are consolidated (weights 1 descriptor each, q/kv as chunk-pairs,
    outputs paired) because each descriptor costs ~630ns on the shared HWDGE;
    queues split SP: q+out, ACT: kv.
Host: out[b] = sum of the 4 cores' fp16->f32 partials + bo.
"""

import os
import sys

import numpy as np

sys.path.insert(0, "/opt/trn_rl_repo")

B, N, E = 2, 2048, 1024
H, D = 16, 64
HPC = 4  # heads per core
DPC = HPC * D  # 256 output dims per core
SCALE = float(E) ** -0.5
KCQ = 8  # contraction chunks for Q/K (no bias row)
KCV = 9  # contraction chunks for V (8x128 data + bias/ones row)
NT = N // 128  # 16 key tiles
W65 = HPC * (D + 1)  # V output width: 4 heads x (64 dims + ones col)
MASK_NEG = np.float32(-60.0)  # masked logit offset (fp16-safe; exp -> 0)
S16 = np.float32(1024.0 / np.log(2.0))  # Schraudolph fp16 scale
B16 = np.float32(15.0 * 1024.0 - 59.3)  # fp16 exponent bias - minimax shift
_CACHED = {}


def build_bass(repeat=1):
    import concourse.bass as bass
    import concourse.mybir as mybir
    import concourse.tile as tile
    from concourse import bacc

    f32 = mybir.dt.float32
    f16 = mybir.dt.float16
    i16 = mybir.dt.int16
    Exp = mybir.ActivationFunctionType.Exp
    Copy = mybir.ActivationFunctionType.Copy
    Alu = mybir.AluOpType

    nc = bacc.Bacc("TRN2", target_bir_lowering=False)

    qT = nc.dram_tensor("qT", (KCQ, 128, N), f16, kind="ExternalInput")
    kvT = nc.dram_tensor("kvT", (KCV, 128, N), f16, kind="ExternalInput")
    wq = nc.dram_tensor("wq", (KCQ, 128, DPC), f16, kind="ExternalInput")
    wk = nc.dram_tensor("wk", (KCQ, 128, DPC), f16, kind="ExternalInput")
    wv = nc.dram_tensor("wv", (KCV, 128, W65), f16, kind="ExternalInput")
    wo = nc.dram_tensor("wo", (2, 128, E), f16, kind="ExternalInput")
    mk = nc.dram_tensor("mk", (128, NT), f32, kind="ExternalInput")
    mk16 = nc.dram_tensor("mk16", (128, NT), f32, kind="ExternalInput")
    bqk = nc.dram_tensor("bqk", (128, 4), f32, kind="ExternalInput")
    outp = nc.dram_tensor("outp", (NT, 128, E), f16, kind="ExternalOutput")

    with tile.TileContext(nc) as tc:
        with (
            tc.tile_pool(name="const", bufs=1) as const,
            tc.tile_pool(name="io", bufs=2) as io_pool,
            tc.tile_pool(name="ups", bufs=10) as upool,
            tc.tile_pool(name="rps", bufs=2) as rpool,
            tc.tile_pool(name="ps", bufs=2, space="PSUM") as ps,
            tc.tile_pool(name="avp", bufs=2, space="PSUM") as avp,
        ):
            wq_sb = const.tile([128, KCQ, DPC], f16, name="wq_sb")
            wk_sb = const.tile([128, KCQ, DPC], f16, name="wk_sb")
            wv_sb = const.tile([128, KCV, W65], f16, name="wv_sb")
            wo_sb = const.tile([128, 2, E], f16, name="wo_sb")
            mk_sb = const.tile([128, NT], f32, name="mk_sb")
            mk16_sb = const.tile([128, NT], f32, name="mk16_sb")
            bqk_sb = const.tile([128, 4], f32, name="bqk_sb")
            qTp = [const.tile([128, 2, N], f16, name=f"qTp{p}") for p in range(4)]
            kvTp = [const.tile([128, 2, N], f16, name=f"kvTp{p}") for p in range(5)]

            def qTs(kc):
                return qTp[kc // 2][:, kc % 2, :]

            def kvTs(kc):
                return kvTp[kc // 2][:, kc % 2, :]
            xqT = const.tile([128, 2, N], f16, name="xqT")
            xkT = const.tile([128, 2, N], f16, name="xkT")
            xv = const.tile([128, NT, W65], f16, name="xv")
            attn = const.tile([128, 2, N], f16, name="attn")
            # raw (pre-normalize) attention accumulators: [qb][j=2*hp+h2]
            avraw = const.tile([65, 2, 4, 1024], f16, name="avraw")
            warm = const.tile([1, 8], f32, name="warm")

            # loop-invariant loads (outside the repeat body), split over the
            # two HWDGE queues; early-needed weights (wq/wk) lead the SP
            # queue so the body's qT/kvT streams aren't stuck behind them
            nc.sync.dma_start(wq_sb[:], wq[:].rearrange("k p d -> p k d"))
            nc.sync.dma_start(bqk_sb[:], bqk[:])
            nc.scalar.dma_start(wk_sb[:], wk[:].rearrange("k p d -> p k d"))
            nc.scalar.dma_start(wv_sb[:], wv[:].rearrange("k p d -> p k d"))
            nc.scalar.dma_start(wo_sb[:], wo[:].rearrange("k p d -> p k d"))
            nc.scalar.dma_start(mk_sb[:], mk[:])
            nc.scalar.dma_start(mk16_sb[:], mk16[:])
            # warm the ACT exp table early so its load overlaps phase A
            nc.vector.memset(warm[:], 0.0)
            nc.scalar.activation(warm[:], warm[:], Exp)

            def body(_iv=None):
                # -------- input DMA: q on the SP queue, kv on the ACT queue
                for p in range(4):
                    nc.sync.dma_start(
                        qTp[p][:], qT[2 * p : 2 * p + 2].rearrange("k p n -> p k n")
                    )
                for p in range(4):
                    nc.scalar.dma_start(
                        kvTp[p][:], kvT[2 * p : 2 * p + 2].rearrange("k p n -> p k n")
                    )
                nc.scalar.dma_start(kvTp[4][:, 0, :], kvT[8])

                # -------- phase A: projections --------
                # Q: xqT[m, q] = (Wq_m^T q)*SCALE + bq_m*SCALE
                # (half 0 here; half 1 is emitted later as PE filler between
                # attention groups)
                def emit_qproj(half, m):
                    qp = ps.tile([128, 1024], f32, tag="ps", name=f"qp{half}{m}")
                    for kc in range(KCQ):
                        for s in range(2):
                            nc.tensor.matmul(
                                qp[:, s * 512 : (s + 1) * 512],
                                wq_sb[:, kc, m * 128 : (m + 1) * 128],
                                qTs(kc)[:, half * 1024 + s * 512 : half * 1024 + (s + 1) * 512],
                                start=(kc == 0),
                                stop=(kc == KCQ - 1),
                            )
                    nc.vector.tensor_scalar(
                        xqT[:, m, half * 1024 : (half + 1) * 1024],
                        qp[:],
                        bqk_sb[:, m : m + 1],
                        None,
                        Alu.add,
                    )

                # Q half0 (ps pool) interleaved per-chunk with K half0 (avp
                # pool, idle during phase A): PE consumes each input chunk as
                # it lands instead of idling through the DMA head
                qps = [ps.tile([128, 1024], f32, tag="ps", name=f"qp0{m}") for m in range(2)]
                kps = [avp.tile([128, 1024], f32, tag="av", name=f"kp0{m}") for m in range(2)]
                for kc in range(KCQ):
                    for m in range(2):
                        for s in range(2):
                            nc.tensor.matmul(
                                qps[m][:, s * 512 : (s + 1) * 512],
                                wq_sb[:, kc, m * 128 : (m + 1) * 128],
                                qTs(kc)[:, s * 512 : (s + 1) * 512],
                                start=(kc == 0),
                                stop=(kc == KCQ - 1),
                            )
                    for m in range(2):
                        for s in range(2):
                            nc.tensor.matmul(
                                kps[m][:, s * 512 : (s + 1) * 512],
                                wk_sb[:, kc, m * 128 : (m + 1) * 128],
                                kvTs(kc)[:, s * 512 : (s + 1) * 512],
                                start=(kc == 0),
                                stop=(kc == KCQ - 1),
                            )
                # xk copies first: they gate the K-half1 psum ring slots
                for m in range(2):
                    nc.vector.tensor_scalar(
                        xkT[:, m, 0:1024], kps[m][:], bqk_sb[:, 2 + m : 3 + m], None, Alu.add
                    )
                for m in range(2):
                    nc.vector.tensor_scalar(
                        xqT[:, m, 0:1024], qps[m][:], bqk_sb[:, m : m + 1], None, Alu.add
                    )
                def emit_qk(qb, hp, kt):
                    qks = []
                    for h2 in range(2):
                        po = 64 * h2
                        qk = ps.tile([128, 1024], f32, tag="ps", name=f"qk{h2}")
                        for s in range(2):
                            nc.tensor.matmul(
                                qk[:, s * 512 : (s + 1) * 512],
                                xkT[po : po + 64, hp, kt * 128 : (kt + 1) * 128],
                                xqT[
                                    po : po + 64,
                                    hp,
                                    qb * 1024 + s * 512 : qb * 1024 + (s + 1) * 512,
                                ],
                                start=True,
                                stop=True,
                            )
                        qks.append(qk)
                    return qks

                def emit_exp(kt, h2, qk):
                    u = upool.tile([128, 1024], f16, tag="u", name=f"u{h2}")
                    if h2 == 1:
                        # Schraudolph: (qk*S16 + mask16) -> i16, bitcast fp16
                        # == exp(qk + mask)
                        nc.vector.tensor_scalar(
                            u[:].bitcast(i16),
                            qk[:],
                            float(S16),
                            mk16_sb[:, kt : kt + 1],
                            Alu.mult,
                            Alu.add,
                        )
                    else:
                        nc.scalar.activation(
                            u[:], qk[:], Exp, bias=mk_sb[:, kt : kt + 1], scale=1.0
                        )
                    return u

                # bank the first attention group's QK+exp before the
                # K-half1 and V projections: the exp engines (the binding
                # resource) work through this otherwise exp-idle PE stretch
                g1_bank = {}
                for _k in range(4):
                    _qk = emit_qk(0, 0, _k)
                    g1_bank[_k] = [emit_exp(_k, h2, _qk[h2]) for h2 in range(2)]

                # K half1 (avp ring again)
                kps1 = [avp.tile([128, 1024], f32, tag="av", name=f"kp1{m}") for m in range(2)]
                for kc in range(KCQ):
                    for m in range(2):
                        for s in range(2):
                            nc.tensor.matmul(
                                kps1[m][:, s * 512 : (s + 1) * 512],
                                wk_sb[:, kc, m * 128 : (m + 1) * 128],
                                kvTs(kc)[:, 1024 + s * 512 : 1024 + (s + 1) * 512],
                                start=(kc == 0),
                                stop=(kc == KCQ - 1),
                            )
                for m in range(2):
                    nc.vector.tensor_scalar(
                        xkT[:, m, 1024:2048], kps1[m][:], bqk_sb[:, 2 + m : 3 + m], None, Alu.add
                    )
                # V (natural layout, 65th column per head = ones for denom)
                for nt in range(NT):
                    vp = ps.tile([128, 1024], f32, tag="ps", name=f"vp{nt}")
                    for kc in range(KCV):
                        nc.tensor.matmul(
                            vp[:, 0:W65],
                            kvTs(kc)[:, nt * 128 : (nt + 1) * 128],
                            wv_sb[:, kc, :],
                            start=(kc == 0),
                            stop=(kc == KCV - 1),
                        )
                    nc.scalar.activation(xv[:, nt, :], vp[:, 0:W65], Copy)

                # -------- phase B: attention (+ interleaved C of prev qb) ----
                c_state = {}

                def emit_c(nt):
                    op = ps.tile([128, 1024], f32, tag="ps", name=f"op{nt}")
                    for c in range(2):
                        for s in range(2):
                            nc.tensor.matmul(
                                op[:, s * 512 : (s + 1) * 512],
                                attn[:, c, nt * 128 : (nt + 1) * 128],
                                wo_sb[:, c, s * 512 : (s + 1) * 512],
                                start=(c == 0),
                                stop=(c == 1),
                            )
                    if nt % 2 == 0:
                        c_state["ot"] = io_pool.tile(
                            [128, 2, 1024], f16, tag="ot", name=f"ot{nt}"
                        )
                    ot = c_state["ot"]
                    # PSUM->SBUF copies round-robin ACT/Pool: both are idle
                    # during C blocks, and DVE must stay clear for the next
                    # group's exp stream (engine queues run in order)
                    if nt % 2 == 0:
                        nc.scalar.activation(ot[:, 0, :], op[:], Copy)
                    else:
                        nc.vector.tensor_copy(ot[:, 1, :], op[:])
                        nc.sync.dma_start(
                            outp[nt - 1 : nt + 1].rearrange("j p e -> p j e"), ot[:]
                        )

                def emit_norm(qb, hp, h2):
                    # lazy softmax normalize from the raw SBUF accumulators
                    j = 2 * hp + h2
                    po = 64 * h2
                    rs = rpool.tile([1, 1024], f16, tag="rs", name="rs")
                    nc.vector.tensor_scalar(
                        rs[:], avraw[64:65, qb, j, :], 1.0, None, Alu.add
                    )
                    with nc.allow_low_precision(reason="softmax denom recip in fp16"):
                        nc.vector.reciprocal(rs[:], rs[:])
                    rb = rpool.tile([64, 1024], f16, tag="rb", name="rb")
                    nc.gpsimd.partition_broadcast(rb[:], rs[0:1, :])
                    # all-fp16 operands -> DVE 2x mode
                    nc.vector.tensor_mul(
                        attn[po : po + 64, hp, qb * 1024 : (qb + 1) * 1024],
                        avraw[0:64, qb, j, :],
                        rb[:],
                    )

                # group schedule: (qb, hp) plus PE-filler ops emitted at the
                # top of each group (before its kt loop, after its prologue
                # QK) so PE never idles while the previous group's PSUM
                # accumulators spill / normalize
                groups = [(0, 0), (0, 1), (1, 0), (1, 1)]
                fillers = {
                    1: lambda: emit_qproj(1, 0),
                    2: lambda: emit_qproj(1, 1),
                    3: lambda: [emit_c(nt) for nt in range(0, 8)],
                }
                prev = None
                for gi, (qb, hp) in enumerate(groups):
                    avs = [
                        avp.tile([128, 1024], f32, tag="av", name=f"av{qb}{hp}{i}")
                        for i in range(2)
                    ]
                    # deep prologue: bank several key-tiles of QK+exp before
                    # the PE filler block so the (binding) exp engines work
                    # through the exp-idle filler window instead of idling;
                    # the u-ring holds the banked tiles
                    P = 4 if gi == 0 else 2
                    us = g1_bank if gi == 0 else {}

                    def emit_kt(kt, qb=qb, hp=hp, us=us):
                        qk = emit_qk(qb, hp, kt)
                        us[kt] = [emit_exp(kt, h2, qk[h2]) for h2 in range(2)]

                    if gi > 0:
                        for k in range(min(P, NT)):
                            emit_kt(k)
                    if gi in fillers:
                        fillers[gi]()
                    for kt in range(NT):
                        if kt + P < NT:
                            emit_kt(kt + P)
                        # previous group's lazy normalize, spread thin
                        if prev is not None and kt in (2, 5):
                            emit_norm(prev[0], prev[1], 0 if kt == 2 else 1)
                        u_cur = us.pop(kt)
                        for h2 in range(2):
                            h = 2 * hp + h2
                            for s in range(2):
                                nc.tensor.matmul(
                                    avs[h2][0:65, s * 512 : (s + 1) * 512],
                                    xv[:, kt, h * 65 : (h + 1) * 65],
                                    u_cur[h2][:, s * 512 : (s + 1) * 512],
                                    start=(kt == 0),
                                    stop=(kt == NT - 1),
                                )
                    # spill raw accumulators to SBUF: frees PSUM banks fast
                    # so the next group's accumulators alloc without waiting
                    # on the (slow) normalize chain
                    nc.vector.tensor_copy(avraw[:, qb, 2 * hp, :], avs[0][0:65, :])
                    nc.scalar.activation(
                        avraw[:, qb, 2 * hp + 1, :], avs[1][0:65, :], Copy
                    )
                    prev = (qb, hp)
                # tail: last group's normalize + last qb's output projection
                for h2 in range(2):
                    emit_norm(1, 1, h2)
                for nt in range(8, 16):
                    emit_c(nt)

            if repeat == 1:
                body()
            else:
                with tc.For_i(0, repeat, 1) as _i:
                    body(_i)

    nc.compile()
    return nc


def make_in_maps(q, kv, mask, Wq, bq, Wkv, bkv, Wo, bo):
    q = np.asarray(q, dtype=np.float32)
    kv = np.asarray(kv, dtype=np.float32)
    mask = np.asarray(mask)
    Wq = np.asarray(Wq, dtype=np.float32)
    bq = np.asarray(bq, dtype=np.float32)
    Wkv = np.asarray(Wkv, dtype=np.float32)
    bkv = np.asarray(bkv, dtype=np.float32)
    Wo = np.asarray(Wo, dtype=np.float32)

    Wk, Wv = Wkv[:E], Wkv[E:]
    bk, bv = bkv[:E], bkv[E:]

    qTa, kvTa, mks, mk16s = {}, {}, {}, {}
    for b in range(B):
        qTa[b] = np.ascontiguousarray(q[b].T).astype(np.float16).reshape(KCQ, 128, N)
        t = np.zeros((KCV * 128, N), np.float16)
        t[:E] = kv[b].T
        t[E] = 1.0
        kvTa[b] = t.reshape(KCV, 128, N)
        mf = np.where(mask[b] == 0, MASK_NEG, mask[b].astype(np.float32))
        mks[b] = np.ascontiguousarray(mf.reshape(NT, 128).T)
        mk16s[b] = np.ascontiguousarray((mf * S16 + B16).reshape(NT, 128).T)

    in_maps = []
    for c in range(8):
        b, g = divmod(c, 4)
        hs = slice(DPC * g, DPC * (g + 1))

        wqa = (Wq[hs].T * SCALE).astype(np.float16).reshape(KCQ, 128, DPC)
        wka = Wk[hs].T.astype(np.float16).reshape(KCQ, 128, DPC)
        # V weights augmented with bias row and per-head ones column
        wva = np.zeros((KCV * 128, W65), np.float16)
        Wv_core = Wv[hs]  # [256, 1024]
        for h in range(HPC):
            wva[:E, h * 65 : h * 65 + 64] = Wv_core[h * 64 : (h + 1) * 64].T
            wva[E, h * 65 : h * 65 + 64] = bv[hs][h * 64 : (h + 1) * 64]
            wva[E, h * 65 + 64] = 1.0
        woT = np.ascontiguousarray(Wo[:, hs].T).astype(np.float16)  # [256, 1024]
        bqka = np.zeros((128, 4), np.float32)
        bqka[:, 0] = bq[hs][0:128] * SCALE
        bqka[:, 1] = bq[hs][128:256] * SCALE
        bqka[:, 2] = bk[hs][0:128]
        bqka[:, 3] = bk[hs][128:256]

        in_maps.append(
            {
                "qT": qTa[b],
                "kvT": kvTa[b],
                "wq": wqa,
                "wk": wka,
                "wv": wva.reshape(KCV, 128, W65),
                "wo": woT.reshape(2, 128, E),
                "mk": mks[b],
                "mk16": mk16s[b],
                "bqk": bqka,
            }
        )
    return in_maps


def kernel(q, kv, mask, Wq, bq, Wkv, bkv, Wo, bo, _repeat=1):
    from concourse.bass_utils import run_bass_kernel_spmd

    key = f"nc_{_repeat}"
    if key not in _CACHED:
        _CACHED[key] = build_bass(repeat=_repeat)
    nc = _CACHED[key]

    in_maps = make_in_maps(q, kv, mask, Wq, bq, Wkv, bkv, Wo, bo)
    res = run_bass_kernel_spmd(nc, in_maps, core_ids=list(range(8)))
    _CACHED["last_result"] = res

    bo = np.asarray(bo, dtype=np.float32)
    outs = [np.asarray(res.results[c]["outp"], np.float32).reshape(N, E) for c in range(8)]
    out = np.stack(
        [
            outs[0] + outs[1] + outs[2] + outs[3],
            outs[4] + outs[5] + outs[6] + outs[7],
        ]
    )
    out += bo[None, None, :]
    return out.astype(np.float32)


# revision 5
# speedup vs baseline: 1.2871x; 1.0432x over previous
"""Trainium2 Bass kernel for nn_MultiHeadAttention_58548994179754 (v2, fp16).

Sharding: 8 cores = 2 batches x 4 head-groups (4 heads x 64 dims per core).

v2 design vs baseline:
  - fp16 datapath end to end (inputs, weights, xq/xk/xv, u=exp(logits), attn):
    halves DMA + SBUF traffic; PE rate identical (1 cycle/row for fp16 at any
    free size, same as f32r at >=256).
  - Q/K projections contract over 8x128 (no bias row); biases are applied by
    the PSUM->SBUF copy (tensor_scalar add with a per-partition bias AP).
    V keeps the 9th (bias) chunk and carries the softmax-denominator ones
    column inside the augmented V weights.
  - exp split 64/64 across ACT (true exp, fp16 out) and DVE (Schraudolph:
    y = x*1477.32 + 15300.7 -> i16, bitcast fp16 == exp(x) to ~3%). HW A/B
    showed ACT exp costs ~1.5us/tile on silicon (vs 1.04 modeled): all-ACT
    measured 432us, 80/48 split 360-383us, 64/64 split 332us. The
    software-pipelined key-tile loop (QK/exp of kt+1 emitted before AV of kt)
    keeps the tensor engine at full p-state clock.
  - attention accumulators spill RAW (pre-normalize) to SBUF fp16 right after
    the last AV matmul, freeing their PSUM banks ~3 us earlier; the softmax
    normalize (recip + partition-broadcast + mul) runs lazily on DVE/Pool
    during the next group's key-tile loop, off the PE critical path.
  - PE-filler blocks at every attention-group boundary (Q-half1 projection
    split over two boundaries, prev-qb output projection at the third, last
    qb's at the tail) keep the PE warm while accumulators spill/normalize.
  - DMAs are consolidated (weights 1 descriptor each, q/kv as chunk-pairs,
    outputs paired) because each descriptor costs ~630ns on the shared HWDGE;
    queues split SP: q+out, ACT: kv.
Host: out[b] = sum of the 4 cores' fp16->f32 partials + bo.
"""

import os
import sys

import numpy as np

sys.path.insert(0, "/opt/trn_rl_repo")

B, N, E = 2, 2048, 1024
H, D = 16, 64
HPC = 4  # heads per core
DPC = HPC * D  # 256 output dims per core
SCALE = float(E) ** -0.5
KCQ = 8  # contraction chunks for Q/K (no bias row)
KCV = 9  # contraction chunks for V (8x128 data + bias/ones row)
NT = N // 128  # 16 key tiles
W65 = HPC * (D + 1)  # V output width: 4 heads x (64 dims + ones col)
MASK_NEG = np.float32(-60.0)  # masked logit offset (fp16-safe; exp -> 0)
S16 = np.float32(1024.0 / np.log(2.0))  # Schraudolph fp16 scale
B16 = np.float32(15.0 * 1024.0 - 59.3)  # fp16 exponent bias - minimax shift
_CACHED = {}


def build_bass(repeat=1):
    import concourse.bass as bass
    import concourse.mybir as mybir
    import concourse.tile as tile
    from concourse import bacc

    f32 = mybir.dt.float32
    f16 = mybir.dt.float16
    i16 = mybir.dt.int16
    Exp = mybir.ActivationFunctionType.Exp
    Copy = mybir.ActivationFunctionType.Copy
    Alu = mybir.AluOpType

    nc = bacc.Bacc("TRN2", target_bir_lowering=False)

    qT = nc.dram_tensor("qT", (KCQ, 128, N), f16, kind="ExternalInput")
    kvT = nc.dram_tensor("kvT", (KCV, 128, N), f16, kind="ExternalInput")
    wq = nc.dram_tensor("wq", (KCQ, 128, DPC), f16, kind="ExternalInput")
    wk = nc.dram_tensor("wk", (KCQ, 128, DPC), f16, kind="ExternalInput")
    wv = nc.dram_tensor("wv", (KCV, 128, W65), f16, kind="ExternalInput")
    wo = nc.dram_tensor("wo", (2, 128, E), f16, kind="ExternalInput")
    mk = nc.dram_tensor("mk", (128, NT), f32, kind="ExternalInput")
    mk16 = nc.dram_tensor("mk16", (128, NT), f32, kind="ExternalInput")
    bqk = nc.dram_tensor("bqk", (128, 4), f32, kind="ExternalInput")
    outp = nc.dram_tensor("outp", (NT, 128, E), f16, kind="ExternalOutput")

    with tile.TileContext(nc) as tc:
        with (
            tc.tile_pool(name="const", bufs=1) as const,
            tc.tile_pool(name="io", bufs=2) as io_pool,
            tc.tile_pool(name="ups", bufs=10) as upool,
            tc.tile_pool(name="rps", bufs=2) as rpool,
            tc.tile_pool(name="ps", bufs=2, space="PSUM") as ps,
            tc.tile_pool(name="avp", bufs=2, space="PSUM") as avp,
        ):
            wq_sb = const.tile([128, KCQ, DPC], f16, name="wq_sb")
            wk_sb = const.tile([128, KCQ, DPC], f16, name="wk_sb")
            wv_sb = const.tile([128, KCV, W65], f16, name="wv_sb")
            wo_sb = const.tile([128, 2, E], f16, name="wo_sb")
            mk_sb = const.tile([128, NT], f32, name="mk_sb")
            mk16_sb = const.tile([128, NT], f32, name="mk16_sb")
            bqk_sb = const.tile([128, 4], f32, name="bqk_sb")
            qTp = [const.tile([128, 2, N], f16, name=f"qTp{p}") for p in range(4)]
            kvTp = [const.tile([128, 2, N], f16, name=f"kvTp{p}") for p in range(5)]

            def qTs(kc):
                return qTp[kc // 2][:, kc % 2, :]

            def kvTs(kc):
                return kvTp[kc // 2][:, kc % 2, :]
            xqT = const.tile([128, 2, N], f16, name="xqT")
            xkT = const.tile([128, 2, N], f16, name="xkT")
            xv = const.tile([128, NT, W65], f16, name="xv")
            attn = const.tile([128, 2, N], f16, name="attn")
            # raw (pre-normalize) attention accumulators: [qb][j=2*hp+h2]
            avraw = const.tile([65, 2, 4, 1024], f16, name="avraw")
            warm = const.tile([1, 8], f32, name="warm")

            # loop-invariant loads (outside the repeat body), split over the
            # two HWDGE queues; early-needed weights (wq/wk) lead the SP
            # queue so the body's qT/kvT streams aren't stuck behind them
            nc.sync.dma_start(wq_sb[:], wq[:].rearrange("k p d -> p k d"))
            nc.sync.dma_start(bqk_sb[:], bqk[:])
            nc.scalar.dma_start(wk_sb[:], wk[:].rearrange("k p d -> p k d"))
            nc.scalar.dma_start(wv_sb[:], wv[:].rearrange("k p d -> p k d"))
            nc.scalar.dma_start(wo_sb[:], wo[:].rearrange("k p d -> p k d"))
            nc.scalar.dma_start(mk_sb[:], mk[:])
            nc.scalar.dma_start(mk16_sb[:], mk16[:])
            # warm the ACT exp table early so its load overlaps phase A
            nc.vector.memset(warm[:], 0.0)
            nc.scalar.activation(warm[:], warm[:], Exp)

            def body(_iv=None):
                # -------- input DMA: q on the SP queue, kv on the ACT queue
                for p in range(4):
                    nc.sync.dma_start(
                        qTp[p][:], qT[2 * p : 2 * p + 2].rearrange("k p n -> p k n")
                    )
                for p in range(4):
                    nc.scalar.dma_start(
                        kvTp[p][:], kvT[2 * p : 2 * p + 2].rearrange("k p n -> p k n")
                    )
                nc.scalar.dma_start(kvTp[4][:, 0, :], kvT[8])

                # -------- phase A: projections --------
                # Q: xqT[m, q] = (Wq_m^T q)*SCALE + bq_m*SCALE
                # (half 0 here; half 1 is emitted later as PE filler between
                # attention groups)
                def emit_qproj(half, m):
                    qp = ps.tile([128, 1024], f32, tag="ps", name=f"qp{half}{m}")
                    for kc in range(KCQ):
                        for s in range(2):
                            nc.tensor.matmul(
                                qp[:, s * 512 : (s + 1) * 512],
                                wq_sb[:, kc, m * 128 : (m + 1) * 128],
                                qTs(kc)[:, half * 1024 + s * 512 : half * 1024 + (s + 1) * 512],
                                start=(kc == 0),
                                stop=(kc == KCQ - 1),
                            )
                    nc.vector.tensor_scalar(
                        xqT[:, m, half * 1024 : (half + 1) * 1024],
                        qp[:],
                        bqk_sb[:, m : m + 1],
                        None,
                        Alu.add,
                    )

                # Q half0 (ps pool) interleaved per-chunk with K half0 (avp
                # pool, idle during phase A): PE consumes each input chunk as
                # it lands instead of idling through the DMA head
                qps = [ps.tile([128, 1024], f32, tag="ps", name=f"qp0{m}") for m in range(2)]
                kps = [avp.tile([128, 1024], f32, tag="av", name=f"kp0{m}") for m in range(2)]
                for kc in range(KCQ):
                    for m in range(2):
                        for s in range(2):
                            nc.tensor.matmul(
                                qps[m][:, s * 512 : (s + 1) * 512],
                                wq_sb[:, kc, m * 128 : (m + 1) * 128],
                                qTs(kc)[:, s * 512 : (s + 1) * 512],
                                start=(kc == 0),
                                stop=(kc == KCQ - 1),
                            )
                    for m in range(2):
                        for s in range(2):
                            nc.tensor.matmul(
                                kps[m][:, s * 512 : (s + 1) * 512],
                                wk_sb[:, kc, m * 128 : (m + 1) * 128],
                                kvTs(kc)[:, s * 512 : (s + 1) * 512],
                                start=(kc == 0),
                                stop=(kc == KCQ - 1),
                            )
                # xk copies first: they gate the K-half1 psum ring slots
                for m in range(2):
                    nc.vector.tensor_scalar(
                        xkT[:, m, 0:1024], kps[m][:], bqk_sb[:, 2 + m : 3 + m], None, Alu.add
                    )
                for m in range(2):
                    nc.vector.tensor_scalar(
                        xqT[:, m, 0:1024], qps[m][:], bqk_sb[:, m : m + 1], None, Alu.add
                    )
                def emit_qk(qb, hp, kt):
                    qks = []
                    for h2 in range(2):
                        po = 64 * h2
                        qk = ps.tile([128, 1024], f32, tag="ps", name=f"qk{h2}")
                        for s in range(2):
                            nc.tensor.matmul(
                                qk[:, s * 512 : (s + 1) * 512],
                                xkT[po : po + 64, hp, kt * 128 : (kt + 1) * 128],
                                xqT[
                                    po : po + 64,
                                    hp,
                                    qb * 1024 + s * 512 : qb * 1024 + (s + 1) * 512,
                                ],
                                start=True,
                                stop=True,
                            )
                        qks.append(qk)
                    return qks

                def emit_exp(kt, h2, qk):
                    u = upool.tile([128, 1024], f16, tag="u", name=f"u{h2}")
                    if h2 == 1:
                        # Schraudolph: (qk*S16 + mask16) -> i16, bitcast fp16
                        # == exp(qk + mask)
                        nc.vector.tensor_scalar(
                            u[:].bitcast(i16),
                            qk[:],
                            float(S16),
                            mk16_sb[:, kt : kt + 1],
                            Alu.mult,
                            Alu.add,
                        )
                    else:
                        nc.scalar.activation(
                            u[:], qk[:], Exp, bias=mk_sb[:, kt : kt + 1], scale=1.0
                        )
                    return u

                # bank the first attention group's QK+exp before the
                # K-half1 and V projections: the exp engines (the binding
                # resource) work through this otherwise exp-idle PE stretch
                g1_bank = {}
                for _k in range(4):
                    _qk = emit_qk(0, 0, _k)
                    g1_bank[_k] = [emit_exp(_k, h2, _qk[h2]) for h2 in range(2)]

                # K half1 (avp ring again)
                kps1 = [avp.tile([128, 1024], f32, tag="av", name=f"kp1{m}") for m in range(2)]
                for kc in range(KCQ):
                    for m in range(2):
                        for s in range(2):
                            nc.tensor.matmul(
                                kps1[m][:, s * 512 : (s + 1) * 512],
                                wk_sb[:, kc, m * 128 : (m + 1) * 128],
                                kvTs(kc)[:, 1024 + s * 512 : 1024 + (s + 1) * 512],
                                start=(kc == 0),
                                stop=(kc == KCQ - 1),
                            )
                for m in range(2):
                    nc.vector.tensor_scalar(
                        xkT[:, m, 1024:2048], kps1[m][:], bqk_sb[:, 2 + m : 3 + m], None, Alu.add
                    )
                # V (natural layout, 65th column per head = ones for denom)
                for nt in range(NT):
                    vp = ps.tile([128, 1024], f32, tag="ps", name=f"vp{nt}")
                    for kc in range(KCV):
                        nc.tensor.matmul(
                            vp[:, 0:W65],
                            kvTs(kc)[:, nt * 128 : (nt + 1) * 128],
                            wv_sb[:, kc, :],
                            start=(kc == 0),
                            stop=(kc == KCV - 1),
                        )
                    if nt % 2 == 0:
                        nc.scalar.activation(xv[:, nt, :], vp[:, 0:W65], Copy)
                    else:
                        nc.vector.tensor_copy(xv[:, nt, :], vp[:, 0:W65])

                # -------- phase B: attention (+ interleaved C of prev qb) ----
                c_state = {}

                def emit_c(nt):
                    op = ps.tile([128, 1024], f32, tag="ps", name=f"op{nt}")
                    for c in range(2):
                        for s in range(2):
                            nc.tensor.matmul(
                                op[:, s * 512 : (s + 1) * 512],
                                attn[:, c, nt * 128 : (nt + 1) * 128],
                                wo_sb[:, c, s * 512 : (s + 1) * 512],
                                start=(c == 0),
                                stop=(c == 1),
                            )
                    if nt % 2 == 0:
                        c_state["ot"] = io_pool.tile(
                            [128, 2, 1024], f16, tag="ot", name=f"ot{nt}"
                        )
                    ot = c_state["ot"]
                    # PSUM->SBUF copies round-robin ACT/Pool: both are idle
                    # during C blocks, and DVE must stay clear for the next
                    # group's exp stream (engine queues run in order)
                    if nt % 2 == 0:
                        nc.scalar.activation(ot[:, 0, :], op[:], Copy)
                    else:
                        nc.vector.tensor_copy(ot[:, 1, :], op[:])
                        nc.sync.dma_start(
                            outp[nt - 1 : nt + 1].rearrange("j p e -> p j e"), ot[:]
                        )

                def emit_norm(qb, hp, h2):
                    # lazy softmax normalize from the raw SBUF accumulators
                    j = 2 * hp + h2
                    po = 64 * h2
                    rs = rpool.tile([1, 1024], f16, tag="rs", name="rs")
                    nc.vector.tensor_scalar(
                        rs[:], avraw[64:65, qb, j, :], 1.0, None, Alu.add
                    )
                    with nc.allow_low_precision(reason="softmax denom recip in fp16"):
                        nc.vector.reciprocal(rs[:], rs[:])
                    rb = rpool.tile([64, 1024], f16, tag="rb", name="rb")
                    nc.gpsimd.partition_broadcast(rb[:], rs[0:1, :])
                    # all-fp16 operands -> DVE 2x mode
                    nc.vector.tensor_mul(
                        attn[po : po + 64, hp, qb * 1024 : (qb + 1) * 1024],
                        avraw[0:64, qb, j, :],
                        rb[:],
                    )

                # group schedule: (qb, hp) plus PE-filler ops emitted at the
                # top of each group (before its kt loop, after its prologue
                # QK) so PE never idles while the previous group's PSUM
                # accumulators spill / normalize
                groups = [(0, 0), (0, 1), (1, 0), (1, 1)]
                fillers = {
                    1: lambda: emit_qproj(1, 0),
                    2: lambda: emit_qproj(1, 1),
                    3: lambda: [emit_c(nt) for nt in range(0, 8)],
                }
                prev = None
                for gi, (qb, hp) in enumerate(groups):
                    avs = [
                        avp.tile([128, 1024], f32, tag="av", name=f"av{qb}{hp}{i}")
                        for i in range(2)
                    ]
                    # deep prologue: bank several key-tiles of QK+exp before
                    # the PE filler block so the (binding) exp engines work
                    # through the exp-idle filler window instead of idling;
                    # the u-ring holds the banked tiles
                    P = 4
                    us = g1_bank if gi == 0 else {}

                    def emit_kt(kt, qb=qb, hp=hp, us=us):
                        qk = emit_qk(qb, hp, kt)
                        us[kt] = [emit_exp(kt, h2, qk[h2]) for h2 in range(2)]

                    if gi > 0:
                        for k in range(min(P, NT)):
                            emit_kt(k)
                    if gi in fillers:
                        fillers[gi]()
                    for kt in range(NT):
                        if kt + P < NT:
                            emit_kt(kt + P)
                        # previous group's lazy normalize, spread thin
                        if prev is not None and kt in (2, 5):
                            emit_norm(prev[0], prev[1], 0 if kt == 2 else 1)
                        u_cur = us.pop(kt)
                        for h2 in range(2):
                            h = 2 * hp + h2
                            for s in range(2):
                                nc.tensor.matmul(
                                    avs[h2][0:65, s * 512 : (s + 1) * 512],
                                    xv[:, kt, h * 65 : (h + 1) * 65],
                                    u_cur[h2][:, s * 512 : (s + 1) * 512],
                                    start=(kt == 0),
                                    stop=(kt == NT - 1),
                                )
                    # spill raw accumulators to SBUF: frees PSUM banks fast
                    # so the next group's accumulators alloc without waiting
                    # on the (slow) normalize chain
                    nc.vector.tensor_copy(avraw[:, qb, 2 * hp, :], avs[0][0:65, :])
                    nc.scalar.activation(
                        avraw[:, qb, 2 * hp + 1, :], avs[1][0:65, :], Copy
                    )
                    prev = (qb, hp)
                # tail: last group's normalize + last qb's output projection
                for h2 in range(2):
                    emit_norm(1, 1, h2)
                for nt in range(8, 16):
                    emit_c(nt)

            if repeat == 1:
                body()
            else:
                with tc.For_i(0, repeat, 1) as _i:
                    body(_i)

    nc.compile()
    return nc


def make_in_maps(q, kv, mask, Wq, bq, Wkv, bkv, Wo, bo):
    q = np.asarray(q, dtype=np.float32)
    kv = np.asarray(kv, dtype=np.float32)
    mask = np.asarray(mask)
    Wq = np.asarray(Wq, dtype=np.float32)
    bq = np.asarray(bq, dtype=np.float32)
    Wkv = np.asarray(Wkv, dtype=np.float32)
    bkv = np.asarray(bkv, dtype=np.float32)
    Wo = np.asarray(Wo, dtype=np.float32)

    Wk, Wv = Wkv[:E], Wkv[E:]
    bk, bv = bkv[:E], bkv[E:]

    qTa, kvTa, mks, mk16s = {}, {}, {}, {}
    for b in range(B):
        qTa[b] = np.ascontiguousarray(q[b].T).astype(np.float16).reshape(KCQ, 128, N)
        t = np.zeros((KCV * 128, N), np.float16)
        t[:E] = kv[b].T
        t[E] = 1.0
        kvTa[b] = t.reshape(KCV, 128, N)
        mf = np.where(mask[b] == 0, MASK_NEG, mask[b].astype(np.float32))
        mks[b] = np.ascontiguousarray(mf.reshape(NT, 128).T)
        mk16s[b] = np.ascontiguousarray((mf * S16 + B16).reshape(NT, 128).T)

    in_maps = []
    for c in range(8):
        b, g = divmod(c, 4)
        hs = slice(DPC * g, DPC * (g + 1))

        wqa = (Wq[hs].T * SCALE).astype(np.float16).reshape(KCQ, 128, DPC)
        wka = Wk[hs].T.astype(np.float16).reshape(KCQ, 128, DPC)
        # V weights augmented with bias row and per-head ones column
        wva = np.zeros((KCV * 128, W65), np.float16)
        Wv_core = Wv[hs]  # [256, 1024]
        for h in range(HPC):
            wva[:E, h * 65 : h * 65 + 64] = Wv_core[h * 64 : (h + 1) * 64].T
            wva[E, h * 65 : h * 65 + 64] = bv[hs][h * 64 : (h + 1) * 64]
            wva[E, h * 65 + 64] = 1.0
        woT = np.ascontiguousarray(Wo[:, hs].T).astype(np.float16)  # [256, 1024]
        bqka = np.zeros((128, 4), np.float32)
        bqka[:, 0] = bq[hs][0:128] * SCALE
        bqka[:, 1] = bq[hs][128:256] * SCALE
        bqka[:, 2] = bk[hs][0:128]
        bqka[:, 3] = bk[hs][128:256]

        in_maps.append(
            {
                "qT": qTa[b],
                "kvT": kvTa[b],
                "wq": wqa,
                "wk": wka,
                "wv": wva.reshape(KCV, 128, W65),
                "wo": woT.reshape(2, 128, E),
                "mk": mks[b],
                "mk16": mk16s[b],
                "bqk": bqka,
            }
        )
    return in_maps


def kernel(q, kv, mask, Wq, bq, Wkv, bkv, Wo, bo, _repeat=1):
    from concourse.bass_utils import run_bass_kernel_spmd

    key = f"nc_{_repeat}"
    if key not in _CACHED:
        _CACHED[key] = build_bass(repeat=_repeat)
    nc = _CACHED[key]

    in_maps = make_in_maps(q, kv, mask, Wq, bq, Wkv, bkv, Wo, bo)
    res = run_bass_kernel_spmd(nc, in_maps, core_ids=list(range(8)))
    _CACHED["last_result"] = res

    bo = np.asarray(bo, dtype=np.float32)
    outs = [np.asarray(res.results[c]["outp"], np.float32).reshape(N, E) for c in range(8)]
    out = np.stack(
        [
            outs[0] + outs[1] + outs[2] + outs[3],
            outs[4] + outs[5] + outs[6] + outs[7],
        ]
    )
    out += bo[None, None, :]
    return out.astype(np.float32)


# revision 6
# speedup vs baseline: 1.3066x; 1.0151x over previous
"""Trainium2 Bass kernel for nn_MultiHeadAttention_58548994179754 (v2, fp16).

Sharding: 8 cores = 2 batches x 4 head-groups (4 heads x 64 dims per core).

v2 design vs baseline:
  - fp16 datapath end to end (inputs, weights, xq/xk/xv, u=exp(logits), attn):
    halves DMA + SBUF traffic; PE rate identical (1 cycle/row for fp16 at any
    free size, same as f32r at >=256).
  - Q/K projections contract over 8x128 (no bias row); biases are applied by
    the PSUM->SBUF copy (tensor_scalar add with a per-partition bias AP).
    V keeps the 9th (bias) chunk and carries the softmax-denominator ones
    column inside the augmented V weights.
  - exp split 64/64 across ACT (true exp, fp16 out) and DVE (Schraudolph:
    y = x*1477.32 + 15300.7 -> i16, bitcast fp16 == exp(x) to ~3%). HW A/B
    showed ACT exp costs ~1.5us/tile on silicon (vs 1.04 modeled): all-ACT
    measured 432us, 80/48 split 360-383us, 64/64 split 332us. The
    software-pipelined key-tile loop (QK/exp of kt+1 emitted before AV of kt)
    keeps the tensor engine at full p-state clock.
  - attention accumulators spill RAW (pre-normalize) to SBUF fp16 right after
    the last AV matmul, freeing their PSUM banks ~3 us earlier; the softmax
    normalize (recip + partition-broadcast + mul) runs lazily on DVE/Pool
    during the next group's key-tile loop, off the PE critical path.
  - PE-filler blocks at every attention-group boundary (Q-half1 projection
    split over two boundaries, prev-qb output projection at the third, last
    qb's at the tail) keep the PE warm while accumulators spill/normalize.
  - DMAs are consolidated (weights 1 descriptor each, q/kv as chunk-pairs,
    outputs paired) because each descriptor costs ~630ns on the shared HWDGE;
    queues split SP: q+out, ACT: kv.
Host: out[b] = sum of the 4 cores' fp16->f32 partials + bo.
"""

import os
import sys

import numpy as np

sys.path.insert(0, "/opt/trn_rl_repo")

B, N, E = 2, 2048, 1024
H, D = 16, 64
HPC = 4  # heads per core
DPC = HPC * D  # 256 output dims per core
SCALE = float(E) ** -0.5
KCQ = 8  # contraction chunks for Q/K (no bias row)
KCV = 9  # contraction chunks for V (8x128 data + bias/ones row)
NT = N // 128  # 16 key tiles
W65 = HPC * (D + 1)  # V output width: 4 heads x (64 dims + ones col)
MASK_NEG = np.float32(-60.0)  # masked logit offset (fp16-safe; exp -> 0)
S16 = np.float32(1024.0 / np.log(2.0))  # Schraudolph fp16 scale
B16 = np.float32(15.0 * 1024.0 - 59.3)  # fp16 exponent bias - minimax shift
_CACHED = {}


def build_bass(repeat=1):
    import concourse.bass as bass
    import concourse.mybir as mybir
    import concourse.tile as tile
    from concourse import bacc

    f32 = mybir.dt.float32
    f16 = mybir.dt.float16
    i16 = mybir.dt.int16
    Exp = mybir.ActivationFunctionType.Exp
    Copy = mybir.ActivationFunctionType.Copy
    Alu = mybir.AluOpType

    nc = bacc.Bacc("TRN2", target_bir_lowering=False)

    qT = nc.dram_tensor("qT", (KCQ, 128, N), f16, kind="ExternalInput")
    kvT = nc.dram_tensor("kvT", (KCV, 128, N), f16, kind="ExternalInput")
    wq = nc.dram_tensor("wq", (KCQ, 128, DPC), f16, kind="ExternalInput")
    wk = nc.dram_tensor("wk", (KCQ, 128, DPC), f16, kind="ExternalInput")
    wv = nc.dram_tensor("wv", (KCV, 128, W65), f16, kind="ExternalInput")
    wo = nc.dram_tensor("wo", (2, 128, E), f16, kind="ExternalInput")
    mk = nc.dram_tensor("mk", (128, NT), f32, kind="ExternalInput")
    mk16 = nc.dram_tensor("mk16", (128, NT), f32, kind="ExternalInput")
    bqk = nc.dram_tensor("bqk", (128, 4), f32, kind="ExternalInput")
    outp = nc.dram_tensor("outp", (NT, 128, E), f16, kind="ExternalOutput")

    with tile.TileContext(nc) as tc:
        with (
            tc.tile_pool(name="const", bufs=1) as const,
            tc.tile_pool(name="io", bufs=2) as io_pool,
            tc.tile_pool(name="ups", bufs=18) as upool,
            tc.tile_pool(name="rps", bufs=2) as rpool,
            tc.tile_pool(name="ps", bufs=2, space="PSUM") as ps,
            tc.tile_pool(name="avp", bufs=2, space="PSUM") as avp,
        ):
            wq_sb = const.tile([128, KCQ, DPC], f16, name="wq_sb")
            wk_sb = const.tile([128, KCQ, DPC], f16, name="wk_sb")
            wv_sb = const.tile([128, KCV, W65], f16, name="wv_sb")
            wo_sb = const.tile([128, 2, E], f16, name="wo_sb")
            mk_sb = const.tile([128, NT], f32, name="mk_sb")
            mk16_sb = const.tile([128, NT], f32, name="mk16_sb")
            bqk_sb = const.tile([128, 4], f32, name="bqk_sb")
            qTp = [const.tile([128, 2, N], f16, name=f"qTp{p}") for p in range(4)]
            kvTp = [const.tile([128, 2, N], f16, name=f"kvTp{p}") for p in range(5)]

            def qTs(kc):
                return qTp[kc // 2][:, kc % 2, :]

            def kvTs(kc):
                return kvTp[kc // 2][:, kc % 2, :]
            xqT = const.tile([128, 2, N], f16, name="xqT")
            xkT = const.tile([128, 2, N], f16, name="xkT")
            xv = const.tile([128, NT, W65], f16, name="xv")
            attn = const.tile([128, 2, N], f16, name="attn")
            # raw (pre-normalize) attention accumulators: [qb][j=2*hp+h2]
            avraw = const.tile([65, 2, 4, 1024], f16, name="avraw")
            warm = const.tile([1, 8], f32, name="warm")

            # loop-invariant loads (outside the repeat body), split over the
            # two HWDGE queues; early-needed weights (wq/wk) lead the SP
            # queue so the body's qT/kvT streams aren't stuck behind them
            nc.sync.dma_start(wq_sb[:], wq[:].rearrange("k p d -> p k d"))
            nc.sync.dma_start(bqk_sb[:], bqk[:])
            nc.scalar.dma_start(wk_sb[:], wk[:].rearrange("k p d -> p k d"))
            nc.scalar.dma_start(wv_sb[:], wv[:].rearrange("k p d -> p k d"))
            nc.scalar.dma_start(wo_sb[:], wo[:].rearrange("k p d -> p k d"))
            nc.scalar.dma_start(mk_sb[:], mk[:])
            nc.scalar.dma_start(mk16_sb[:], mk16[:])
            # warm the ACT exp table early so its load overlaps phase A
            nc.vector.memset(warm[:], 0.0)
            nc.scalar.activation(warm[:], warm[:], Exp)

            def body(_iv=None):
                # -------- input DMA: q on the SP queue, kv on the ACT queue
                for p in range(4):
                    nc.sync.dma_start(
                        qTp[p][:], qT[2 * p : 2 * p + 2].rearrange("k p n -> p k n")
                    )
                for p in range(4):
                    nc.scalar.dma_start(
                        kvTp[p][:], kvT[2 * p : 2 * p + 2].rearrange("k p n -> p k n")
                    )
                nc.scalar.dma_start(kvTp[4][:, 0, :], kvT[8])

                # -------- phase A: projections --------
                # Q: xqT[m, q] = (Wq_m^T q)*SCALE + bq_m*SCALE
                # (half 0 here; half 1 is emitted later as PE filler between
                # attention groups)
                def emit_qproj(half, m):
                    qp = ps.tile([128, 1024], f32, tag="ps", name=f"qp{half}{m}")
                    for kc in range(KCQ):
                        for s in range(2):
                            nc.tensor.matmul(
                                qp[:, s * 512 : (s + 1) * 512],
                                wq_sb[:, kc, m * 128 : (m + 1) * 128],
                                qTs(kc)[:, half * 1024 + s * 512 : half * 1024 + (s + 1) * 512],
                                start=(kc == 0),
                                stop=(kc == KCQ - 1),
                            )
                    nc.vector.tensor_scalar(
                        xqT[:, m, half * 1024 : (half + 1) * 1024],
                        qp[:],
                        bqk_sb[:, m : m + 1],
                        None,
                        Alu.add,
                    )

                # Q half0 (ps pool) interleaved per-chunk with K half0 (avp
                # pool, idle during phase A): PE consumes each input chunk as
                # it lands instead of idling through the DMA head
                qps = [ps.tile([128, 1024], f32, tag="ps", name=f"qp0{m}") for m in range(2)]
                kps = [avp.tile([128, 1024], f32, tag="av", name=f"kp0{m}") for m in range(2)]
                for kc in range(KCQ):
                    for m in range(2):
                        for s in range(2):
                            nc.tensor.matmul(
                                qps[m][:, s * 512 : (s + 1) * 512],
                                wq_sb[:, kc, m * 128 : (m + 1) * 128],
                                qTs(kc)[:, s * 512 : (s + 1) * 512],
                                start=(kc == 0),
                                stop=(kc == KCQ - 1),
                            )
                    for m in range(2):
                        for s in range(2):
                            nc.tensor.matmul(
                                kps[m][:, s * 512 : (s + 1) * 512],
                                wk_sb[:, kc, m * 128 : (m + 1) * 128],
                                kvTs(kc)[:, s * 512 : (s + 1) * 512],
                                start=(kc == 0),
                                stop=(kc == KCQ - 1),
                            )
                # xk copies first: they gate the K-half1 psum ring slots
                for m in range(2):
                    nc.vector.tensor_scalar(
                        xkT[:, m, 0:1024], kps[m][:], bqk_sb[:, 2 + m : 3 + m], None, Alu.add
                    )
                for m in range(2):
                    nc.vector.tensor_scalar(
                        xqT[:, m, 0:1024], qps[m][:], bqk_sb[:, m : m + 1], None, Alu.add
                    )
                def emit_qk(qb, hp, kt):
                    qks = []
                    for h2 in range(2):
                        po = 64 * h2
                        qk = ps.tile([128, 1024], f32, tag="ps", name=f"qk{h2}")
                        for s in range(2):
                            nc.tensor.matmul(
                                qk[:, s * 512 : (s + 1) * 512],
                                xkT[po : po + 64, hp, kt * 128 : (kt + 1) * 128],
                                xqT[
                                    po : po + 64,
                                    hp,
                                    qb * 1024 + s * 512 : qb * 1024 + (s + 1) * 512,
                                ],
                                start=True,
                                stop=True,
                            )
                        qks.append(qk)
                    return qks

                def emit_exp(kt, h2, qk):
                    u = upool.tile([128, 1024], f16, tag="u", name=f"u{h2}")
                    if h2 == 1:
                        # Schraudolph: (qk*S16 + mask16) -> i16, bitcast fp16
                        # == exp(qk + mask)
                        nc.vector.tensor_scalar(
                            u[:].bitcast(i16),
                            qk[:],
                            float(S16),
                            mk16_sb[:, kt : kt + 1],
                            Alu.mult,
                            Alu.add,
                        )
                    else:
                        nc.scalar.activation(
                            u[:], qk[:], Exp, bias=mk_sb[:, kt : kt + 1], scale=1.0
                        )
                    return u

                # bank the first attention group's QK+exp through the
                # K-half1 and V projections: the exp engines (the binding
                # resource) work through this otherwise exp-idle PE stretch.
                # Emitted progressively (2 key-tiles at a time) so the V
                # copies never queue behind long exp runs on ACT/DVE.
                g1_bank = {}

                def bank_g1(k0):
                    for _k in (k0, k0 + 1):
                        _qk = emit_qk(0, 0, _k)
                        g1_bank[_k] = [emit_exp(_k, h2, _qk[h2]) for h2 in range(2)]

                bank_g1(0)
                # K half1 (avp ring again)
                kps1 = [avp.tile([128, 1024], f32, tag="av", name=f"kp1{m}") for m in range(2)]
                for kc in range(KCQ):
                    for m in range(2):
                        for s in range(2):
                            nc.tensor.matmul(
                                kps1[m][:, s * 512 : (s + 1) * 512],
                                wk_sb[:, kc, m * 128 : (m + 1) * 128],
                                kvTs(kc)[:, 1024 + s * 512 : 1024 + (s + 1) * 512],
                                start=(kc == 0),
                                stop=(kc == KCQ - 1),
                            )
                for m in range(2):
                    nc.vector.tensor_scalar(
                        xkT[:, m, 1024:2048], kps1[m][:], bqk_sb[:, 2 + m : 3 + m], None, Alu.add
                    )
                bank_g1(2)
                # V (natural layout, 65th column per head = ones for denom)
                for nt in range(NT):
                    if nt == 6:
                        bank_g1(4)
                    elif nt == 11:
                        bank_g1(6)
                    vp = ps.tile([128, 1024], f32, tag="ps", name=f"vp{nt}")
                    for kc in range(KCV):
                        nc.tensor.matmul(
                            vp[:, 0:W65],
                            kvTs(kc)[:, nt * 128 : (nt + 1) * 128],
                            wv_sb[:, kc, :],
                            start=(kc == 0),
                            stop=(kc == KCV - 1),
                        )
                    if nt % 2 == 0:
                        nc.scalar.activation(xv[:, nt, :], vp[:, 0:W65], Copy)
                    else:
                        nc.vector.tensor_copy(xv[:, nt, :], vp[:, 0:W65])

                # -------- phase B: attention (+ interleaved C of prev qb) ----
                c_state = {}

                def emit_c(nt):
                    op = ps.tile([128, 1024], f32, tag="ps", name=f"op{nt}")
                    for c in range(2):
                        for s in range(2):
                            nc.tensor.matmul(
                                op[:, s * 512 : (s + 1) * 512],
                                attn[:, c, nt * 128 : (nt + 1) * 128],
                                wo_sb[:, c, s * 512 : (s + 1) * 512],
                                start=(c == 0),
                                stop=(c == 1),
                            )
                    if nt % 2 == 0:
                        c_state["ot"] = io_pool.tile(
                            [128, 2, 1024], f16, tag="ot", name=f"ot{nt}"
                        )
                    ot = c_state["ot"]
                    # PSUM->SBUF copies round-robin ACT/Pool: both are idle
                    # during C blocks, and DVE must stay clear for the next
                    # group's exp stream (engine queues run in order)
                    if nt % 2 == 0:
                        nc.scalar.activation(ot[:, 0, :], op[:], Copy)
                    else:
                        nc.vector.tensor_copy(ot[:, 1, :], op[:])
                        nc.sync.dma_start(
                            outp[nt - 1 : nt + 1].rearrange("j p e -> p j e"), ot[:]
                        )

                def emit_norm(qb, hp, h2):
                    # lazy softmax normalize from the raw SBUF accumulators
                    j = 2 * hp + h2
                    po = 64 * h2
                    rs = rpool.tile([1, 1024], f16, tag="rs", name="rs")
                    nc.vector.tensor_scalar(
                        rs[:], avraw[64:65, qb, j, :], 1.0, None, Alu.add
                    )
                    with nc.allow_low_precision(reason="softmax denom recip in fp16"):
                        nc.vector.reciprocal(rs[:], rs[:])
                    rb = rpool.tile([64, 1024], f16, tag="rb", name="rb")
                    nc.gpsimd.partition_broadcast(rb[:], rs[0:1, :])
                    # all-fp16 operands -> DVE 2x mode
                    nc.vector.tensor_mul(
                        attn[po : po + 64, hp, qb * 1024 : (qb + 1) * 1024],
                        avraw[0:64, qb, j, :],
                        rb[:],
                    )

                # group schedule: (qb, hp) plus PE-filler ops emitted at the
                # top of each group (before its kt loop, after its prologue
                # QK) so PE never idles while the previous group's PSUM
                # accumulators spill / normalize
                groups = [(0, 0), (0, 1), (1, 0), (1, 1)]
                fillers = {
                    1: lambda: emit_qproj(1, 0),
                    2: lambda: emit_qproj(1, 1),
                    3: lambda: [emit_c(nt) for nt in range(0, 8)],
                }
                prev = None
                for gi, (qb, hp) in enumerate(groups):
                    avs = [
                        avp.tile([128, 1024], f32, tag="av", name=f"av{qb}{hp}{i}")
                        for i in range(2)
                    ]
                    # deep prologue: bank several key-tiles of QK+exp before
                    # the PE filler block so the (binding) exp engines work
                    # through the exp-idle filler window instead of idling;
                    # the u-ring holds the banked tiles
                    P = 8 if gi == 0 else 4
                    us = g1_bank if gi == 0 else {}

                    def emit_kt(kt, qb=qb, hp=hp, us=us):
                        qk = emit_qk(qb, hp, kt)
                        us[kt] = [emit_exp(kt, h2, qk[h2]) for h2 in range(2)]

                    if gi > 0:
                        for k in range(min(P, NT)):
                            emit_kt(k)
                    if gi in fillers:
                        fillers[gi]()
                    for kt in range(NT):
                        if kt + P < NT:
                            emit_kt(kt + P)
                        # previous group's lazy normalize, spread thin
                        if prev is not None and kt in (2, 5):
                            emit_norm(prev[0], prev[1], 0 if kt == 2 else 1)
                        u_cur = us.pop(kt)
                        for h2 in range(2):
                            h = 2 * hp + h2
                            for s in range(2):
                                nc.tensor.matmul(
                                    avs[h2][0:65, s * 512 : (s + 1) * 512],
                                    xv[:, kt, h * 65 : (h + 1) * 65],
                                    u_cur[h2][:, s * 512 : (s + 1) * 512],
                                    start=(kt == 0),
                                    stop=(kt == NT - 1),
                                )
                    # spill raw accumulators to SBUF: frees PSUM banks fast
                    # so the next group's accumulators alloc without waiting
                    # on the (slow) normalize chain
                    nc.vector.tensor_copy(avraw[:, qb, 2 * hp, :], avs[0][0:65, :])
                    nc.scalar.activation(
                        avraw[:, qb, 2 * hp + 1, :], avs[1][0:65, :], Copy
                    )
                    prev = (qb, hp)
                # tail: last group's normalize + last qb's output projection
                for h2 in range(2):
                    emit_norm(1, 1, h2)
                for nt in range(8, 16):
                    emit_c(nt)

            if repeat == 1:
                body()
            else:
                with tc.For_i(0, repeat, 1) as _i:
                    body(_i)

    nc.compile()
    return nc


def make_in_maps(q, kv, mask, Wq, bq, Wkv, bkv, Wo, bo):
    q = np.asarray(q, dtype=np.float32)
    kv = np.asarray(kv, dtype=np.float32)
    mask = np.asarray(mask)
    Wq = np.asarray(Wq, dtype=np.float32)
    bq = np.asarray(bq, dtype=np.float32)
    Wkv = np.asarray(Wkv, dtype=np.float32)
    bkv = np.asarray(bkv, dtype=np.float32)
    Wo = np.asarray(Wo, dtype=np.float32)

    Wk, Wv = Wkv[:E], Wkv[E:]
    bk, bv = bkv[:E], bkv[E:]

    qTa, kvTa, mks, mk16s = {}, {}, {}, {}
    for b in range(B):
        qTa[b] = np.ascontiguousarray(q[b].T).astype(np.float16).reshape(KCQ, 128, N)
        t = np.zeros((KCV * 128, N), np.float16)
        t[:E] = kv[b].T
        t[E] = 1.0
        kvTa[b] = t.reshape(KCV, 128, N)
        mf = np.where(mask[b] == 0, MASK_NEG, mask[b].astype(np.float32))
        mks[b] = np.ascontiguousarray(mf.reshape(NT, 128).T)
        mk16s[b] = np.ascontiguousarray((mf * S16 + B16).reshape(NT, 128).T)

    in_maps = []
    for c in range(8):
        b, g = divmod(c, 4)
        hs = slice(DPC * g, DPC * (g + 1))

        wqa = (Wq[hs].T * SCALE).astype(np.float16).reshape(KCQ, 128, DPC)
        wka = Wk[hs].T.astype(np.float16).reshape(KCQ, 128, DPC)
        # V weights augmented with bias row and per-head ones column
        wva = np.zeros((KCV * 128, W65), np.float16)
        Wv_core = Wv[hs]  # [256, 1024]
        for h in range(HPC):
            wva[:E, h * 65 : h * 65 + 64] = Wv_core[h * 64 : (h + 1) * 64].T
            wva[E, h * 65 : h * 65 + 64] = bv[hs][h * 64 : (h + 1) * 64]
            wva[E, h * 65 + 64] = 1.0
        woT = np.ascontiguousarray(Wo[:, hs].T).astype(np.float16)  # [256, 1024]
        bqka = np.zeros((128, 4), np.float32)
        bqka[:, 0] = bq[hs][0:128] * SCALE
        bqka[:, 1] = bq[hs][128:256] * SCALE
        bqka[:, 2] = bk[hs][0:128]
        bqka[:, 3] = bk[hs][128:256]

        in_maps.append(
            {
                "qT": qTa[b],
                "kvT": kvTa[b],
                "wq": wqa,
                "wk": wka,
                "wv": wva.reshape(KCV, 128, W65),
                "wo": woT.reshape(2, 128, E),
                "mk": mks[b],
                "mk16": mk16s[b],
                "bqk": bqka,
            }
        )
    return in_maps


def kernel(q, kv, mask, Wq, bq, Wkv, bkv, Wo, bo, _repeat=1):
    from concourse.bass_utils import run_bass_kernel_spmd

    key = f"nc_{_repeat}"
    if key not in _CACHED:
        _CACHED[key] = build_bass(repeat=_repeat)
    nc = _CACHED[key]

    in_maps = make_in_maps(q, kv, mask, Wq, bq, Wkv, bkv, Wo, bo)
    res = run_bass_kernel_spmd(nc, in_maps, core_ids=list(range(8)))
    _CACHED["last_result"] = res

    bo = np.asarray(bo, dtype=np.float32)
    outs = [np.asarray(res.results[c]["outp"], np.float32).reshape(N, E) for c in range(8)]
    out = np.stack(
        [
            outs[0] + outs[1] + outs[2] + outs[3],
            outs[4] + outs[5] + outs[6] + outs[7],
        ]
    )
    out += bo[None, None, :]
    return out.astype(np.float32)
